# revision 69
# baseline (speedup 1.0000x reference)
"""Trainium2 Bass kernel for a local-attention block (MQA, RoPE, causal mask).

Reference computation (B=2, T=2048, WIDTH=2560, 10 q-heads, 1 kv-head,
head_dim=256, window=2048 => mask reduces to causal & same-segment):

    q = x @ wq.T ; k = x @ wk.T ; v = x @ wv.T
    q, k = rope(q), rope(k)
    probs = softmax(q k^T / 16 + mask)
    out = (probs @ v) @ w_final.T + b_final

Sharding: 8 cores = 2 batches x 4 interleaved query-subtile sets. Core
(b, s) owns q-subtiles {s, s+4, s+8, s+12} (128 tokens each) of batch b.
Slot j (subtile s+4j) needs only the causal k-tile prefix 0..4j+3 — a
FIXED tile count 4*(j+1) on every core, so the device program is
SPMD-uniform while each core skips the k-tiles causality forbids
(400 vs 640 [128x128] attention tile-units per core; ideal is 340).
The <=3-tile overshoot past the causal diagonal is zeroed by a
data-driven 0/1 mask applied only to the last 4-tile group of each
slot; earlier groups are entirely inside the causal region (single
segment; segment_pos is arange per the problem spec).

Precision strategy: projections run as fp8e4m3 DoubleRow matmuls
(0.5 cycles/row) with a host-side hi/lo split of both operands and
three bilinear terms (x_hi*w_hi + x_hi*w_lo + x_lo*w_hi); the dropped
x_lo*w_lo term is ~0.1%. QK^T runs the same 3-term fp8 scheme on
rope'd q/k; P@V runs fp16.
"""

import sys

import numpy as np

for _p in ("/opt/trn_rl_repo", "/root/.axon_site/_ro/trn_rl_repo"):
    if _p not in sys.path:
        sys.path.insert(0, _p)

import ml_dtypes

FP8 = ml_dtypes.float8_e4m3
FP16 = np.float16

B, T, WIDTH = 2, 2048, 2560
NUM_HEADS, HEAD_DIM = 10, 256
WINDOW = 2048
MAX_WAVELENGTH = 10000.0
QBLK = 512              # query tokens per core (4 subtiles of 128)
NW = WIDTH // 128       # 20 width stripes
NKP = NW // 2           # 10 DoubleRow contraction pairs
NTT = T // 128          # 16 token tiles
NQS = QBLK // 128       # 4 query subtiles (slots)
VROW = HEAD_DIM + 1     # v columns + ones column (denominator trick)
# fp8 pre-scales: lift x/w out of the fp8 subnormal floor before hi/lo
# quantization; the product is descaled by DESCALE on eviction.
S_X, S_W = 8.0, 128.0
DESCALE = 1.0 / (S_X * S_W)
S_QK = 8.0              # extra q/k scale lifting their fp8 lo parts
SCL_EVICT = S_QK * DESCALE

_NC_CACHE = {}


def _build_nc():
    """Build the (single, SPMD-uniform) Bass/Tile program."""
    import concourse.bass as bass  # noqa: F401
    import concourse.mybir as mybir
    import concourse.tile as tile
    from concourse import bacc
    from concourse.masks import make_identity

    fp32 = mybir.dt.float32
    fp16 = mybir.dt.float16
    fp8 = mybir.dt.float8e4
    Exp = mybir.ActivationFunctionType.Exp
    Ident = mybir.ActivationFunctionType.Identity
    Mult = mybir.AluOpType.mult
    Sub = mybir.AluOpType.subtract
    DR = mybir.MatmulPerfMode.DoubleRow

    nc = bacc.Bacc("TRN2", target_bir_lowering=False, debug=False)

    # ---- DRAM I/O ----
    # xq*: x^T columns gathered at the core's 4 q-subtiles (hi/lo fp8).
    # xf*: full x^T in natural token order (hi/lo fp8).
    xqh = nc.dram_tensor("xqh", [NW, 128, QBLK], fp8, kind="ExternalInput")
    xql = nc.dram_tensor("xql", [NW, 128, QBLK], fp8, kind="ExternalInput")
    xfh = nc.dram_tensor("xfh", [NW, 128, T], fp8, kind="ExternalInput")
    xfl = nc.dram_tensor("xfl", [NW, 128, T], fp8, kind="ExternalInput")
    wq = nc.dram_tensor("wq", [NW, 2, 128, WIDTH], fp8, kind="ExternalInput")
    wk = nc.dram_tensor("wk", [2, 128, NW * HEAD_DIM], fp8, kind="ExternalInput")
    wv = nc.dram_tensor("wv", [2, 128, NW * HEAD_DIM], fp8, kind="ExternalInput")
    wf = nc.dram_tensor("wf", [NW, 2, 128, WIDTH], fp8, kind="ExternalInput")
    # Cos/sin rope tables, each duplicated across both partition halves
    # (SBUF TensorTensor requires equal input base partitions); cols
    # 0:QBLK q positions (gathered), QBLK: natural k positions.
    trc = nc.dram_tensor("trc", [128, QBLK + T], fp16, kind="ExternalInput")
    trs = nc.dram_tensor("trs", [128, QBLK + T], fp16, kind="ExternalInput")
    # msk: [128 k, (j*4+tt)*128 + c] = 0/1 mask for slot j, group tile
    # 4j+tt, q col c of subtile j.
    msk = nc.dram_tensor("msk", [128, NQS * QBLK], fp16, kind="ExternalInput")
    bia = nc.dram_tensor("bia", [128, NW], fp32, kind="ExternalInput")
    out = nc.dram_tensor("out", [NW, 128, QBLK], fp16, kind="ExternalOutput")

    with tile.TileContext(nc) as tc:
        with (
            tc.tile_pool(name="res", bufs=1) as res,
            tc.tile_pool(name="bigA", bufs=1) as bigA,
            tc.tile_pool(name="bigB", bufs=1) as bigB,
            tc.tile_pool(name="wqs", bufs=4) as wqs,
            tc.tile_pool(name="ptp", bufs=8) as ptp,
            tc.tile_pool(name="enp", bufs=2) as enp,
            tc.tile_pool(name="tmp", bufs=1) as tmpp,
            tc.tile_pool(name="rcp", bufs=2) as rcpp,
            tc.tile_pool(name="outp", bufs=2) as outp,
            tc.tile_pool(name="stp", bufs=4, space="PSUM") as stp,
            tc.tile_pool(name="op", bufs=4, space="PSUM") as op,
        ):
            # ---- resident SBUF tiles ----
            xq8 = res.tile([128, NW, QBLK], fp8, tag="xq8")      # q-col x^T hi
            xqlo = res.tile([128, NW, QBLK], fp8, tag="xqlo")    # q-col x^T lo
            qtr = res.tile([128, 2, NW, QBLK], fp8, tag="qtr")   # rope'd Q^T hi/lo
            ktr = res.tile([128, 2, 2, T], fp8, tag="ktr")       # rope'd K^T hi/lo
            vsb = res.tile([128, NTT * VROW], fp16, tag="vsb")   # V + ones col
            wkr = res.tile([128, 2, NW, HEAD_DIM], fp8, tag="wkr")
            wvr = res.tile([128, 2, NW, HEAD_DIM], fp8, tag="wvr")
            trgc = res.tile([128, QBLK + T], fp16, tag="trgc")
            trgs = res.tile([128, QBLK + T], fp16, tag="trgs")
            masks = res.tile([128, NQS * QBLK], fp16, tag="msk")
            bia_s = res.tile([128, NW], fp32, tag="bia")
            ident = res.tile([128, 128], fp8, tag="ident")
            ones8 = res.tile([128, 1], fp16, tag="ones8")
            ones1 = res.tile([1, 128], fp16, tag="ones1")
            ident32 = res.tile([128, 128], fp32, tag="ident32")

            make_identity(nc, ident[:])
            make_identity(nc, ident32[:])
            nc.gpsimd.memset(ones8[:], 0.125)  # 1/8: bakes enc pre-scale
            nc.gpsimd.memset(ones1[:], 1.0)

            # x^T hi stripes (natural order, full T)
            x8 = bigA.tile([128, NW, T], fp8, tag="bigA")
            # x^T lo stripes; slot later reused for enc^T
            xl = bigB.tile([128, NW, T], fp8, tag="bigB")

            # Fence helper target: a dummy Pool copy keyed on a wq tile
            # paces the bulk Pool-queue loads behind the wq stream.
            dum = res.tile([1, 4], fp8, tag="dum")

            # xq streams in 4 batched halves on the SP queue (HWDGE
            # dispatch is 625ns per DMA); the wq weight stream follows on
            # SP / early stripes on Act so neither blocks the other.
            for half in range(2):  # batched: HWDGE dispatch is 625ns/DMA
                nc.sync.dma_start(
                    out=xq8[:, 10 * half:10 * half + 10, :],
                    in_=xqh[10 * half:10 * half + 10].rearrange(
                        "n p m -> p n m"))
            for half in range(2):
                nc.sync.dma_start(
                    out=xqlo[:, 10 * half:10 * half + 10, :],
                    in_=xql[10 * half:10 * half + 10].rearrange(
                        "n p m -> p n m"))

            wq_tiles = {}

            def issue_wq(m):
                t = wqs.tile([128, 2, NW, 128], fp8, tag="wq")
                # The first stripes ride the Act queue (SP is busy with the
                # xq transfers they must not wait behind); later stripes
                # ride SP, which is free after xq and has no compute ops
                # gating its queue head (Act-queue HOL collapses prefetch).
                eng = nc.scalar if m < 4 else nc.sync
                for s in range(2):  # hi first: first matmuls need only hi
                    eng.dma_start(
                        out=t[:, s].rearrange("p n m -> p (n m)"),
                        in_=wq[m, s])
                wq_tiles[m] = t

            issue_wq(0)
            issue_wq(1)
            # Bulk loads ride the Pool SWDGE queue, split into fenced
            # groups keyed on wq-stripe arrival so they never starve the
            # latency-critical Q-phase weight stream on the shared DMA pool.
            def xf_chunk(hilo, c, half):
                """One [10 stripes, 512 cols] chunk of xfh/xfl (0.66 MB)."""
                a, b = QBLK * c, QBLK * (c + 1)
                n0, n1 = (10, NW) if half else (0, 10)
                src, dst = ((xfh, x8), (xfl, xl))[hilo]
                nc.gpsimd.dma_start(
                    out=dst[:, n0:n1, a:b],
                    in_=src[n0:n1, :, a:b].rearrange("n p m -> p n m"))

            def bulk_group(g):
                if g == 0:
                    nc.gpsimd.dma_start(out=trgc[:], in_=trc[:])
                    nc.gpsimd.dma_start(out=trgs[:], in_=trs[:])
                elif g <= 2:   # xfh col-chunk 0
                    xf_chunk(0, 0, g - 1)
                elif g <= 4:   # xfl col-chunk 0
                    xf_chunk(1, 0, g - 3)
                elif g == 5:   # wk hi half
                    nc.gpsimd.dma_start(
                        out=wkr[:, 0].rearrange("p n m -> p (n m)"),
                        in_=wk[0])
                elif g == 6:   # wk lo half (needed by K iter 1's 3rd term)
                    nc.gpsimd.dma_start(
                        out=wkr[:, 1].rearrange("p n m -> p (n m)"),
                        in_=wk[1])
                else:          # xfh col-chunk 1, first half
                    xf_chunk(0, 1, 0)

            # Only chunks needed at K-phase start are fenced through the
            # Q loop (the Q phase is otherwise DMA-oversubscribed); the
            # c2/c3 chunks stream during the K loop, whose group order is
            # arranged to consume c0/c1 first.
            _fences = {m: m for m in range(8)}

            # denominator columns of V (softmax denom via matmul); 1/8 so
            # the reciprocal bakes in the fp8 enc pre-scale of 8
            for t in range(NTT):
                nc.gpsimd.memset(vsb[:, t * VROW + HEAD_DIM: (t + 1) * VROW], 0.125)

            def rope_evict(ps, cols, hi0, lo0, hi1, lo1):
                """[hi+lo](0) = ps0*cos - ps1*sin ; (1) = ps1*cos + ps0*sin.

                ps: [128, n] PSUM fp32; cols: slice into the trgc/trgs
                tables (carrying the fp8 descale); hi*/lo*: fp8 SBUF APs.
                A single Act cast to fp16 SBUF frees the PSUM slot fast
                (the ps-pool rotation otherwise stalls the PE) and gives
                every DVE mul 2x 16-bit throughput + the cheaper SBUF
                access latency. Each mul pairs inputs from the SAME base
                partition (hw TensorTensor constraint). The hi/lo split
                runs on the otherwise-idle Pool engine."""
                n = cols.stop - cols.start
                pb = tmpp.tile([128, QBLK], fp16, tag="pb", name="pb")
                ta = tmpp.tile([128, QBLK], fp16, tag="ta", name="ta")
                tb = tmpp.tile([128, QBLK], fp16, tag="tb", name="tb")
                s = tmpp.tile([128, QBLK], fp16, tag="s", name="s")
                nc.scalar.activation(pb[:, :n], ps[:], Ident)
                nc.vector.tensor_mul(ta[0:64, :n], pb[0:64, :n],
                                     trgc[0:64, cols])
                nc.vector.tensor_mul(tb[0:64, :n], pb[64:128, :n],
                                     trgs[64:128, cols])
                nc.vector.tensor_sub(s[0:64, :n], ta[0:64, :n], tb[0:64, :n])
                nc.gpsimd.tensor_copy(hi0, s[0:64, :n])
                nc.gpsimd.tensor_sub(lo0, s[0:64, :n], hi0)
                nc.vector.tensor_mul(ta[64:128, :n], pb[64:128, :n],
                                     trgc[64:128, cols])
                nc.vector.tensor_mul(tb[64:128, :n], pb[0:64, :n],
                                     trgs[0:64, cols])
                nc.vector.tensor_add(s[64:128, :n], ta[64:128, :n],
                                     tb[64:128, :n])
                nc.gpsimd.tensor_copy(hi1, s[64:128, :n])
                nc.gpsimd.tensor_sub(lo1, s[64:128, :n], hi1)

            def split_evict(ps, hi, lo):
                """hi+lo (fp8) = ps * SCL_EVICT, split across ACT and DVE."""
                nc.scalar.activation(hi, ps, Ident, scale=SCL_EVICT)
                nc.vector.scalar_tensor_tensor(
                    lo, ps, SCL_EVICT, hi, Mult, Sub)

            def proj3(ps, whi, wlo, xhi_ap, xlo_ap, wlo_last=False):
                """ps += 3-term hi/lo fp8 DoubleRow product (contraction WIDTH).

                whi/wlo/xhi_ap/xlo_ap: [128, NW, F] fp8 APs (k-stripe dim 2nd).
                wlo_last orders the lo-weight term last so a still-streaming
                lo-weight load has maximum slack."""
                if wlo_last:
                    terms = ((whi, xhi_ap), (whi, xlo_ap), (wlo, xhi_ap))
                else:
                    terms = ((whi, xhi_ap), (wlo, xhi_ap), (whi, xlo_ap))
                for ti, (wt, xt) in enumerate(terms):
                    for kk in range(NKP):
                        nc.tensor.matmul(
                            ps,
                            lhsT=wt[:, 2 * kk:2 * kk + 2, :],
                            rhs=xt[:, 2 * kk:2 * kk + 2, :],
                            start=(ti == 0 and kk == 0),
                            stop=(ti == 2 and kk == NKP - 1),
                            perf_mode=DR,
                        )

            _ps_pools = [(stp, "st"), (op, "o"), (stp, "st"), (op, "o"),
                         (stp, "st"), (op, "o"), (stp, "st"), (op, "o")]

            def proj_ps(i, cols=QBLK):
                pool, tag = _ps_pools[i % len(_ps_pools)]
                return pool.tile([128, cols], fp32, tag=tag, name=f"ps{i}")

            # ---- Q projection -> rope'd Q^T stripes [qdim, QBLK] ----
            # stripe m: qdim rows [128m, 128m+128) = head m//2, half m%2
            # Only stripes 0..7 (heads 0-3) run up front; the rest stream
            # interleaved into the attention pipeline, where the PE has
            # cover while their wq weights arrive (the projection head of
            # the kernel is DMA-bandwidth-bound, the attention window is
            # DMA-idle).
            def q_stripe(m):
                if m + 2 < NW:
                    issue_wq(m + 2)
                wq_m = wq_tiles.pop(m)
                if m in _fences:
                    nc.gpsimd.tensor_copy(dum[:], wq_m[0:1, 0, 0, 0:4])
                    bulk_group(_fences[m])
                ps = proj_ps(m)
                proj3(ps[:], wq_m[:, 0], wq_m[:, 1], xq8[:], xqlo[:])
                if m % 2 == 0:  # rope half of the head dims
                    rope_evict(ps, slice(0, QBLK),
                               qtr[0:64, 0, m, :], qtr[0:64, 1, m, :],
                               qtr[64:128, 0, m, :], qtr[64:128, 1, m, :])
                else:           # passthrough half
                    split_evict(ps[:], qtr[:, 0, m, :], qtr[:, 1, m, :])

            for m in range(NW):
                q_stripe(m)

            # ---- K projection -> rope'd K^T [2, 128, T] fp16 ----
            # Group order consumes col-chunks 0,1 first so the c2/c3
            # loads issued here have time to land.
            _k_iters = [(0, 0), (1, 0), (0, 1), (1, 1),
                        (0, 2), (1, 2), (0, 3), (1, 3)]
            _k_dma = {1: [(1, 1, 0), (1, 1, 1)],
                      2: [(0, 2, 0), (0, 2, 1)], 3: [(1, 2, 0), (1, 2, 1)],
                      4: [(0, 3, 0), (0, 3, 1)], 5: [(1, 3, 0), (1, 3, 1)]}
            for ki, (hh, g) in enumerate(_k_iters):
                    if ki == 0:
                        xf_chunk(0, 1, 1)
                    for ch in _k_dma.get(ki, ()):
                        xf_chunk(*ch)
                    if ki == 6:
                        nc.gpsimd.dma_start(
                            out=wvr[:].rearrange("p s n m -> p s (n m)"),
                            in_=wv[:].rearrange("s p m -> p s m"))
                    elif ki == 7:
                        nc.gpsimd.dma_start(out=masks[:], in_=msk[:])
                        nc.gpsimd.dma_start(out=bia_s[:], in_=bia[:])
                    ps = proj_ps(NW + 4 * hh + g)
                    cols = slice(g * QBLK, (g + 1) * QBLK)
                    kcols = slice(QBLK + g * QBLK, QBLK + (g + 1) * QBLK)
                    proj3(ps[:],
                          wkr[:, 0, :, hh * 128:hh * 128 + 128],
                          wkr[:, 1, :, hh * 128:hh * 128 + 128],
                          x8[:, :, cols], xl[:, :, cols], wlo_last=True)
                    if hh == 0:
                        rope_evict(ps, kcols,
                                   ktr[0:64, 0, 0, cols], ktr[0:64, 1, 0, cols],
                                   ktr[64:128, 0, 0, cols],
                                   ktr[64:128, 1, 0, cols])
                    else:
                        split_evict(ps[:], ktr[:, 0, 1, cols],
                                    ktr[:, 1, 1, cols])

            # ---- V projection: x_hi*wv_hi + x_hi*wv_lo + x_lo*wv_hi ----
            for mt in range(NTT):
                ps = proj_ps(NW + 8 + mt, cols=HEAD_DIM)
                toks = slice(mt * 128, (mt + 1) * 128)
                terms = ((x8, 0), (x8, 1), (xl, 0))
                for ti, (xt, s) in enumerate(terms):
                    for kk in range(NKP):
                        nc.tensor.matmul(
                            ps[:],
                            lhsT=xt[:, 2 * kk:2 * kk + 2, toks],
                            rhs=wvr[:, s, 2 * kk:2 * kk + 2, :],
                            start=(ti == 0 and kk == 0),
                            stop=(ti == 2 and kk == NKP - 1),
                            perf_mode=DR,
                        )
                nc.scalar.activation(
                    vsb[:, mt * VROW: mt * VROW + HEAD_DIM], ps[:], Ident,
                    scale=DESCALE)

            # enc^T (scaled x8, hi/lo fp8) reuses xl's slot
            enct = bigB.tile([128, 2, NW, QBLK], fp8, tag="bigB")

            # ---- attention ----
            # Slot j = q-subtile j (token subtile s+4j): k-tile prefix
            # 0..4j+3 in (j+1) groups of 4 tiles. S^T layout per group:
            # st[:, tt*128+c] = score(k=(4g+tt)*128+p, q=subtile_j col c).
            # Depth-1 software pipeline: QK+exp of slot n is emitted
            # before P@V of slot n-1 so the in-order PE queue always has
            # ready matmuls while exp/mask of the newest slot are in
            # flight on Act/DVE.
            # Head-paired slot order: each j=3 slot sits between j=2/j=3
            # neighbours so its exp chain drains under a long QK+PV cover
            # (a j=0 neighbour would leave the PE waiting ~1us on Act).
            slots = []
            for a in range(NUM_HEADS // 2):
                hA, hB = 2 * a, 2 * a + 1
                slots += [(hA, 2), (hB, 3), (hA, 3), (hB, 2),
                          (hA, 0), (hB, 1), (hA, 1), (hB, 0)]
            pts = {}

            def emit_qk(h, j):
                pt_l = []
                for g in range(j + 1):
                    st = stp.tile([128, QBLK], fp32, tag="st")
                    for tt in range(4):
                        for ti, (ql, kl) in enumerate(
                                ((0, 0), (0, 1), (1, 0))):
                            nc.tensor.matmul(
                                st[:, tt * 128:(tt + 1) * 128],
                                lhsT=ktr[:, kl, 0:2,
                                         (4 * g + tt) * 128:
                                         (4 * g + tt + 1) * 128],
                                rhs=qtr[:, ql, 2 * h:2 * h + 2,
                                        j * 128:(j + 1) * 128],
                                start=(ti == 0), stop=(ti == 2),
                                perf_mode=DR)
                    pt = ptp.tile([128, QBLK], fp16, tag="pt")
                    # p = exp(s / sqrt(head_dim)), masked entries -> 0
                    nc.scalar.activation(pt[:], st[:], Exp,
                                         scale=0.0625 / (S_QK * S_QK))
                    if g == j:  # only the diagonal group needs masking
                        nc.vector.tensor_mul(
                            pt[:], pt[:], masks[:, j * QBLK:(j + 1) * QBLK])
                    pt_l.append(pt)
                pts[(h, j)] = pt_l

            def emit_pv(h, j):
                # Transposed P@V: out^T[hd, q] accumulates directly in the
                # enc^T orientation (lhsT = V tile, rhs = P^T tile), so no
                # PE transposes or PSUM-eviction copies are needed later.
                # One 2KB PSUM bank per slot: enc^T halves [0:256], the
                # denominator column [256] (near-free 1-col matmuls), the
                # transposed 1/denom row [257:385] and its broadcast
                # [384:512]. NOTE: the three accumulation chains must each
                # run their matmuls consecutively — interleaving open
                # accumulation groups within one PSUM bank corrupts them.
                o = op.tile([128, 4 * 128], fp32, tag="o", name=f"o{h}_{j}")
                pt_l = pts.pop((h, j))
                for s2 in range(2):
                    for g in range(j + 1):
                        pt = pt_l[g]
                        for tt in range(4):
                            t = 4 * g + tt
                            nc.tensor.matmul(
                                o[:, s2 * 128:(s2 + 1) * 128],
                                lhsT=vsb[:, t * VROW + s2 * 128:
                                         t * VROW + (s2 + 1) * 128],
                                rhs=pt[:, tt * 128:(tt + 1) * 128],
                                start=(g == 0 and tt == 0),
                                stop=(g == j and tt == 3))
                for g in range(j + 1):
                    pt = pt_l[g]
                    for tt in range(4):
                        nc.tensor.matmul(
                            o[:, 256:257],
                            lhsT=pt[:, tt * 128:(tt + 1) * 128],
                            rhs=ones8[:],
                            start=(g == 0 and tt == 0),
                            stop=(g == j and tt == 3))
                r = rcpp.tile([128, 1], fp32, tag="r")
                nc.vector.reciprocal(r[:], o[:, 256:257])
                ors[(h, j)] = (o, r)

            def emit_rt(h, j):
                # stage n-2: transpose 1/denom to a row + stage it in SBUF
                o, r = ors[(h, j)]
                nc.tensor.matmul(o[0:1, 257:385], lhsT=r[:],
                                 rhs=ident32[:], is_transpose=True)
                rts = rcpp.tile([1, 128], fp16, tag="rts", name=f"rs{h}{j}")
                nc.vector.tensor_copy(rts[:], o[0:1, 257:385])
                ors[(h, j)] = (o, rts)

            def emit_fin(h, j):
                # stage n-3: broadcast 1/denom across partitions, scale,
                # and hi/lo fp8 split straight into enc^T.
                o, rts = ors.pop((h, j))
                nc.tensor.matmul(o[:, 384:512], lhsT=ones1[:], rhs=rts[:])
                # TensorTensor may read only one PSUM input (hw verifier):
                # stage the broadcast through SBUF before the scale.
                rbs = enp.tile([128, 128], fp16, tag="rbs", name=f"rb{h}{j}")
                nc.vector.tensor_copy(rbs[:], o[:, 384:512])
                en = enp.tile([128, 2, 128], fp16, tag="en")
                for s2 in range(2):
                    nc.vector.tensor_mul(
                        en[:, s2, :], o[:, s2 * 128:(s2 + 1) * 128],
                        rbs[:])
                # hi/lo fp8 split on Pool, written directly into enc^T
                eh = enct[:, 0, 2 * h:2 * h + 2, j * 128:(j + 1) * 128]
                nc.gpsimd.tensor_copy(eh, en[:])
                nc.gpsimd.tensor_sub(
                    enct[:, 1, 2 * h:2 * h + 2, j * 128:(j + 1) * 128],
                    en[:], eh)

            ors = {}
            wf_tiles = []

            def issue_wf(m):
                wf_m = wqs.tile([128, 2, NW, 128], fp8, tag="wq",
                                name=f"wf{m}")
                nc.sync.dma_start(
                    out=wf_m[:].rearrange("p s n m -> p s (n m)"),
                    in_=wf[m].rearrange("s p m -> p s m"))
                wf_tiles.append(wf_m)

            for n in range(len(slots) + 4):
                if n == len(slots) - 4:
                    # head start on the wf stream while the attention
                    # tail drains (the wqs pool slots are free by now)
                    for m in range(2):
                        issue_wf(m)
                if n < len(slots):
                    emit_qk(*slots[n])
                if 1 <= n <= len(slots):
                    emit_pv(*slots[n - 1])
                if 2 <= n <= len(slots) + 1:
                    emit_rt(*slots[n - 2])
                if 3 <= n <= len(slots) + 2:
                    emit_fin(*slots[n - 3])

            # ---- final projection: out^T = wf @ enc^T + bias ----
            # Pre-issue the whole wf stream on the Pool SWDGE queue; the
            # 4-deep tile pool lets DMAs run ahead of consumption.
            for m in range(2, NW):
                issue_wf(m)
            for m in range(NW):
                wf_m = wf_tiles[m]
                ps = proj_ps(m + 1)
                for ti, (whl, ehl) in enumerate(((0, 0), (1, 0), (0, 1))):
                    for kk in range(NKP):
                        nc.tensor.matmul(
                            ps[:],
                            lhsT=wf_m[:, whl, 2 * kk:2 * kk + 2, :],
                            rhs=enct[:, ehl, 2 * kk:2 * kk + 2, :],
                            start=(ti == 0 and kk == 0),
                            stop=(ti == 2 and kk == NKP - 1),
                            perf_mode=DR,
                        )
                osb = outp.tile([128, QBLK], fp16, tag="osb")
                nc.scalar.activation(osb[:], ps[:], Ident, scale=DESCALE,
                                     bias=bia_s[:, m:m + 1])
                nc.sync.dma_start(out=out[m], in_=osb[:])

    if not nc.is_finalized():
        nc.finalize()  # bacc register allocation — required before walrus compile
    return nc


def get_nc():
    if "nc" not in _NC_CACHE:
        _NC_CACHE["nc"] = _build_nc()
    return _NC_CACHE["nc"]


def _host_prepare(x, segment_pos, wq, wk, wv, w_final, b_final):
    """Build shared + per-core device input arrays."""
    x = np.asarray(x, dtype=np.float32)
    segment_pos = np.asarray(segment_pos)
    wq = np.asarray(wq, dtype=np.float32)
    wk = np.asarray(wk, dtype=np.float32)
    wv = np.asarray(wv, dtype=np.float32)
    w_final = np.asarray(w_final, dtype=np.float32)
    b_final = np.asarray(b_final, dtype=np.float32)

    def hilo(a, s):
        a = a * s
        hi = a.astype(FP8)
        lo = (a - hi.astype(np.float32)).astype(FP8)
        return hi, lo

    def stripes_sq(w):  # [WIDTH, WIDTH] -> [NW,128,WIDTH] w^T stripes (fp32)
        wt = np.ascontiguousarray(w.T)
        return np.ascontiguousarray(
            wt.reshape(NW, 128, NW, 128).transpose(2, 1, 0, 3).reshape(
                NW, 128, WIDTH))

    def skinny(wt):  # [WIDTH, HEAD_DIM] w^T -> [128, NW*HEAD_DIM] (fp32)
        return np.ascontiguousarray(
            wt.reshape(NW, 128, HEAD_DIM).transpose(1, 0, 2).reshape(
                128, NW * HEAD_DIM))

    wq_hi, wq_lo = hilo(stripes_sq(wq), S_W)
    wk_hi, wk_lo = hilo(skinny(np.ascontiguousarray(wk.T)), S_W)
    wv_hi, wv_lo = hilo(skinny(np.ascontiguousarray(wv.T)), S_W)

    shared = {
        "wq": np.ascontiguousarray(np.stack([wq_hi, wq_lo], axis=1)),
        "wk": np.ascontiguousarray(np.stack([wk_hi, wk_lo], axis=0)),
        "wv": np.ascontiguousarray(np.stack([wv_hi, wv_lo], axis=0)),
        "wf": np.ascontiguousarray(np.stack(
            hilo(stripes_sq(w_final), S_W), axis=1)),
        "bia": np.ascontiguousarray(b_final.reshape(NW, 128).T).astype(np.float32),
    }

    inv_freq = (
        1.0 / MAX_WAVELENGTH ** (2.0 * np.arange(HEAD_DIM // 4, dtype=np.float32)
                                 / (HEAD_DIM // 2))
    ).astype(np.float32)

    in_maps = []
    for c in range(8):
        b = c // 4
        s = c % 4
        # core's q tokens: subtiles {s, s+4, s+8, s+12}, 128 each
        qtok = (np.arange(NQS)[:, None] * 4 + s) * 128 + np.arange(128)[None, :]
        qtok = qtok.reshape(-1)  # [QBLK]

        xT = np.ascontiguousarray(x[b].T)  # [WIDTH, T] fp32
        xT_hi, xT_lo = hilo(xT, S_X)
        xfh_ = np.ascontiguousarray(xT_hi.reshape(NW, 128, T))
        xfl_ = np.ascontiguousarray(xT_lo.reshape(NW, 128, T))

        pos = segment_pos[b].astype(np.float32)
        ang_k = inv_freq[:, None] * pos[None, :]          # [64, T]
        ang_q = ang_k[:, qtok]                            # [64, QBLK]
        ang = np.concatenate([ang_q, ang_k], axis=1)      # [64, QBLK+T]
        trc_ = np.concatenate([np.cos(ang)] * 2, axis=0) * SCL_EVICT
        trs_ = np.concatenate([np.sin(ang)] * 2, axis=0) * SCL_EVICT

        # masks for the diagonal 4-tile group of each slot j: k-tiles
        # 4j..4j+3 vs q-subtile s+4j. allow = causal & window & same-seg.
        seg = np.cumsum((segment_pos[b] == 0).astype(np.int64))
        mask_ = np.zeros((128, NQS * QBLK), dtype=np.float32)
        for j in range(NQS):
            tq = (s + 4 * j) * 128 + np.arange(128)       # [128] q tokens
            for tt in range(4):
                tk = (4 * j + tt) * 128 + np.arange(128)  # [128] k tokens
                allow = (
                    (tk[:, None] <= tq[None, :])
                    & (tq[None, :] <= tk[:, None] + WINDOW)
                    & (seg[tk][:, None] == seg[tq][None, :])
                )
                mask_[:, (j * 4 + tt) * 128:(j * 4 + tt + 1) * 128] = allow

        in_maps.append(dict(
            shared,
            xqh=np.ascontiguousarray(xfh_[:, :, qtok]),
            xql=np.ascontiguousarray(xfl_[:, :, qtok]),
            xfh=xfh_,
            xfl=xfl_,
            trc=trc_.astype(FP16),
            trs=trs_.astype(FP16),
            msk=mask_.astype(FP16),
        ))
    return in_maps


def _assemble(results):
    out = np.empty((B, T, WIDTH), dtype=np.float32)
    for c, res in enumerate(results):
        b, s = c // 4, c % 4
        o = np.asarray(res["out"], dtype=np.float32)  # [NW, 128, QBLK]
        o = o.transpose(2, 0, 1).reshape(QBLK, WIDTH)  # [QBLK, WIDTH]
        for j in range(NQS):
            t0 = (s + 4 * j) * 128
            out[b, t0:t0 + 128, :] = o[j * 128:(j + 1) * 128]
    return out


def kernel(x, segment_pos, wq, wk, wv, w_final, b_final):
    from concourse.bass_utils import run_bass_kernel_spmd

    nc = get_nc()
    in_maps = _host_prepare(x, segment_pos, wq, wk, wv, w_final, b_final)
    res = run_bass_kernel_spmd(nc, in_maps, list(range(8)))
    return _assemble(res.results)


# revision 72
# speedup vs baseline: 1.0058x; 1.0058x over previous
"""Trainium2 Bass kernel for a local-attention block (MQA, RoPE, causal mask).

Reference computation (B=2, T=2048, WIDTH=2560, 10 q-heads, 1 kv-head,
head_dim=256, window=2048 => mask reduces to causal & same-segment):

    q = x @ wq.T ; k = x @ wk.T ; v = x @ wv.T
    q, k = rope(q), rope(k)
    probs = softmax(q k^T / 16 + mask)
    out = (probs @ v) @ w_final.T + b_final

Sharding: 8 cores = 2 batches x 4 interleaved query-subtile sets. Core
(b, s) owns q-subtiles {s, s+4, s+8, s+12} (128 tokens each) of batch b.
Slot j (subtile s+4j) needs only the causal k-tile prefix 0..4j+3 — a
FIXED tile count 4*(j+1) on every core, so the device program is
SPMD-uniform while each core skips the k-tiles causality forbids
(400 vs 640 [128x128] attention tile-units per core; ideal is 340).
The <=3-tile overshoot past the causal diagonal is zeroed by a
data-driven 0/1 mask applied only to the last 4-tile group of each
slot; earlier groups are entirely inside the causal region (single
segment; segment_pos is arange per the problem spec).

Precision strategy: projections run as fp8e4m3 DoubleRow matmuls
(0.5 cycles/row) with a host-side hi/lo split of both operands and
three bilinear terms (x_hi*w_hi + x_hi*w_lo + x_lo*w_hi); the dropped
x_lo*w_lo term is ~0.1%. QK^T runs the same 3-term fp8 scheme on
rope'd q/k; P@V runs fp16.
"""

import sys

import numpy as np

for _p in ("/opt/trn_rl_repo", "/root/.axon_site/_ro/trn_rl_repo"):
    if _p not in sys.path:
        sys.path.insert(0, _p)

import ml_dtypes

FP8 = ml_dtypes.float8_e4m3
FP16 = np.float16

B, T, WIDTH = 2, 2048, 2560
NUM_HEADS, HEAD_DIM = 10, 256
WINDOW = 2048
MAX_WAVELENGTH = 10000.0
QBLK = 512              # query tokens per core (4 subtiles of 128)
NW = WIDTH // 128       # 20 width stripes
NKP = NW // 2           # 10 DoubleRow contraction pairs
NTT = T // 128          # 16 token tiles
NQS = QBLK // 128       # 4 query subtiles (slots)
VROW = HEAD_DIM + 1     # v columns + ones column (denominator trick)
# fp8 pre-scales: lift x/w out of the fp8 subnormal floor before hi/lo
# quantization; the product is descaled by DESCALE on eviction.
S_X, S_W = 8.0, 128.0
DESCALE = 1.0 / (S_X * S_W)
S_QK = 8.0              # extra q/k scale lifting their fp8 lo parts
SCL_EVICT = S_QK * DESCALE

_NC_CACHE = {}


def _build_nc():
    """Build the (single, SPMD-uniform) Bass/Tile program."""
    import concourse.bass as bass  # noqa: F401
    import concourse.mybir as mybir
    import concourse.tile as tile
    from concourse import bacc
    from concourse.masks import make_identity

    fp32 = mybir.dt.float32
    fp16 = mybir.dt.float16
    fp8 = mybir.dt.float8e4
    Exp = mybir.ActivationFunctionType.Exp
    Ident = mybir.ActivationFunctionType.Identity
    Mult = mybir.AluOpType.mult
    Sub = mybir.AluOpType.subtract
    DR = mybir.MatmulPerfMode.DoubleRow

    nc = bacc.Bacc("TRN2", target_bir_lowering=False, debug=False)

    # ---- DRAM I/O ----
    # xq*: x^T columns gathered at the core's 4 q-subtiles (hi/lo fp8).
    # xf*: full x^T in natural token order (hi/lo fp8).
    xqh = nc.dram_tensor("xqh", [NW, 128, QBLK], fp8, kind="ExternalInput")
    xql = nc.dram_tensor("xql", [NW, 128, QBLK], fp8, kind="ExternalInput")
    xfh = nc.dram_tensor("xfh", [NW, 128, T], fp8, kind="ExternalInput")
    xfl = nc.dram_tensor("xfl", [NW, 128, T], fp8, kind="ExternalInput")
    wq = nc.dram_tensor("wq", [NW, 2, 128, WIDTH], fp8, kind="ExternalInput")
    wk = nc.dram_tensor("wk", [2, 128, NW * HEAD_DIM], fp8, kind="ExternalInput")
    wv = nc.dram_tensor("wv", [2, 128, NW * HEAD_DIM], fp8, kind="ExternalInput")
    wf = nc.dram_tensor("wf", [NW, 2, 128, WIDTH], fp8, kind="ExternalInput")
    # Cos/sin rope tables, each duplicated across both partition halves
    # (SBUF TensorTensor requires equal input base partitions); cols
    # 0:QBLK q positions (gathered), QBLK: natural k positions.
    trc = nc.dram_tensor("trc", [128, QBLK + T], fp16, kind="ExternalInput")
    trs = nc.dram_tensor("trs", [128, QBLK + T], fp16, kind="ExternalInput")
    # msk: [128 k, (j*4+tt)*128 + c] = 0/1 mask for slot j, group tile
    # 4j+tt, q col c of subtile j.
    msk = nc.dram_tensor("msk", [128, NQS * QBLK], fp16, kind="ExternalInput")
    bia = nc.dram_tensor("bia", [128, NW], fp32, kind="ExternalInput")
    out = nc.dram_tensor("out", [NW, 128, QBLK], fp16, kind="ExternalOutput")

    with tile.TileContext(nc) as tc:
        with (
            tc.tile_pool(name="res", bufs=1) as res,
            tc.tile_pool(name="bigA", bufs=1) as bigA,
            tc.tile_pool(name="bigB", bufs=1) as bigB,
            tc.tile_pool(name="wqs", bufs=4) as wqs,
            tc.tile_pool(name="ptp", bufs=8) as ptp,
            tc.tile_pool(name="enp", bufs=2) as enp,
            tc.tile_pool(name="tmp", bufs=1) as tmpp,
            tc.tile_pool(name="rcp", bufs=2) as rcpp,
            tc.tile_pool(name="outp", bufs=2) as outp,
            tc.tile_pool(name="stp", bufs=4, space="PSUM") as stp,
            tc.tile_pool(name="op", bufs=4, space="PSUM") as op,
        ):
            # ---- resident SBUF tiles ----
            xq8 = res.tile([128, NW, QBLK], fp8, tag="xq8")      # q-col x^T hi
            xqlo = res.tile([128, NW, QBLK], fp8, tag="xqlo")    # q-col x^T lo
            qtr = res.tile([128, 2, NW, QBLK], fp8, tag="qtr")   # rope'd Q^T hi/lo
            ktr = res.tile([128, 2, 2, T], fp8, tag="ktr")       # rope'd K^T hi/lo
            vsb = res.tile([128, NTT * VROW], fp16, tag="vsb")   # V + ones col
            wkr = res.tile([128, 2, NW, HEAD_DIM], fp8, tag="wkr")
            wvr = res.tile([128, 2, NW, HEAD_DIM], fp8, tag="wvr")
            trgc = res.tile([128, QBLK + T], fp16, tag="trgc")
            trgs = res.tile([128, QBLK + T], fp16, tag="trgs")
            masks = res.tile([128, NQS * QBLK], fp16, tag="msk")
            bia_s = res.tile([128, NW], fp32, tag="bia")
            ident = res.tile([128, 128], fp8, tag="ident")
            ones8 = res.tile([128, 1], fp16, tag="ones8")
            ones1 = res.tile([1, 128], fp16, tag="ones1")
            ident32 = res.tile([128, 128], fp32, tag="ident32")

            make_identity(nc, ident[:])
            make_identity(nc, ident32[:])
            nc.gpsimd.memset(ones8[:], 0.125)  # 1/8: bakes enc pre-scale
            nc.gpsimd.memset(ones1[:], 1.0)

            # x^T hi stripes (natural order, full T)
            x8 = bigA.tile([128, NW, T], fp8, tag="bigA")
            # x^T lo stripes; slot later reused for enc^T
            xl = bigB.tile([128, NW, T], fp8, tag="bigB")

            # Fence helper target: a dummy Pool copy keyed on a wq tile
            # paces the bulk Pool-queue loads behind the wq stream.
            dum = res.tile([1, 4], fp8, tag="dum")

            # xq streams in 4 batched halves on the SP queue (HWDGE
            # dispatch is 625ns per DMA); the wq weight stream follows on
            # SP / early stripes on Act so neither blocks the other.
            for half in range(2):  # batched: HWDGE dispatch is 625ns/DMA
                nc.sync.dma_start(
                    out=xq8[:, 10 * half:10 * half + 10, :],
                    in_=xqh[10 * half:10 * half + 10].rearrange(
                        "n p m -> p n m"))
            for half in range(2):
                nc.sync.dma_start(
                    out=xqlo[:, 10 * half:10 * half + 10, :],
                    in_=xql[10 * half:10 * half + 10].rearrange(
                        "n p m -> p n m"))

            wq_tiles = {}

            def issue_wq(m):
                t = wqs.tile([128, 2, NW, 128], fp8, tag="wq")
                # The first stripes ride the Act queue (SP is busy with the
                # xq transfers they must not wait behind); later stripes
                # ride SP, which is free after xq and has no compute ops
                # gating its queue head (Act-queue HOL collapses prefetch).
                eng = nc.scalar if m < 4 else nc.sync
                for s in range(2):  # hi first: first matmuls need only hi
                    eng.dma_start(
                        out=t[:, s].rearrange("p n m -> p (n m)"),
                        in_=wq[m, s])
                wq_tiles[m] = t

            issue_wq(0)
            issue_wq(1)
            # Bulk loads ride the Pool SWDGE queue, split into fenced
            # groups keyed on wq-stripe arrival so they never starve the
            # latency-critical Q-phase weight stream on the shared DMA pool.
            def xf_chunk(hilo, c, half):
                """One [10 stripes, 512 cols] chunk of xfh/xfl (0.66 MB)."""
                a, b = QBLK * c, QBLK * (c + 1)
                n0, n1 = (10, NW) if half else (0, 10)
                src, dst = ((xfh, x8), (xfl, xl))[hilo]
                nc.gpsimd.dma_start(
                    out=dst[:, n0:n1, a:b],
                    in_=src[n0:n1, :, a:b].rearrange("n p m -> p n m"))

            def bulk_group(g):
                if g == 0:
                    nc.gpsimd.dma_start(out=trgc[:], in_=trc[:])
                    nc.gpsimd.dma_start(out=trgs[:], in_=trs[:])
                elif g <= 2:   # xfh col-chunk 0
                    xf_chunk(0, 0, g - 1)
                elif g <= 4:   # xfl col-chunk 0
                    xf_chunk(1, 0, g - 3)
                elif g == 5:   # wk hi half
                    nc.gpsimd.dma_start(
                        out=wkr[:, 0].rearrange("p n m -> p (n m)"),
                        in_=wk[0])
                elif g == 6:   # wk lo half (needed by K iter 1's 3rd term)
                    nc.gpsimd.dma_start(
                        out=wkr[:, 1].rearrange("p n m -> p (n m)"),
                        in_=wk[1])
                else:          # xfh col-chunk 1, first half
                    xf_chunk(0, 1, 0)

            # Only chunks needed at K-phase start are fenced through the
            # Q loop (the Q phase is otherwise DMA-oversubscribed); the
            # c2/c3 chunks stream during the K loop, whose group order is
            # arranged to consume c0/c1 first.
            _fences = {m: m for m in range(8)}

            # denominator columns of V (softmax denom via matmul); 1/8 so
            # the reciprocal bakes in the fp8 enc pre-scale of 8
            for t in range(NTT):
                nc.gpsimd.memset(vsb[:, t * VROW + HEAD_DIM: (t + 1) * VROW], 0.125)

            def rope_evict(ps, cols, hi0, lo0, hi1, lo1):
                """[hi+lo](0) = ps0*cos - ps1*sin ; (1) = ps1*cos + ps0*sin.

                ps: [128, n] PSUM fp32; cols: slice into the trgc/trgs
                tables (carrying the fp8 descale); hi*/lo*: fp8 SBUF APs.
                A single Act cast to fp16 SBUF frees the PSUM slot fast
                (the ps-pool rotation otherwise stalls the PE) and gives
                every DVE mul 2x 16-bit throughput + the cheaper SBUF
                access latency. Each mul pairs inputs from the SAME base
                partition (hw TensorTensor constraint). The hi/lo split
                runs on the otherwise-idle Pool engine."""
                n = cols.stop - cols.start
                pb = tmpp.tile([128, QBLK], fp16, tag="pb", name="pb")
                ta = tmpp.tile([128, QBLK], fp16, tag="ta", name="ta")
                tb = tmpp.tile([128, QBLK], fp16, tag="tb", name="tb")
                s = tmpp.tile([128, QBLK], fp16, tag="s", name="s")
                nc.scalar.activation(pb[:, :n], ps[:], Ident)
                nc.vector.tensor_mul(ta[0:64, :n], pb[0:64, :n],
                                     trgc[0:64, cols])
                nc.vector.tensor_mul(tb[0:64, :n], pb[64:128, :n],
                                     trgs[64:128, cols])
                nc.vector.tensor_sub(s[0:64, :n], ta[0:64, :n], tb[0:64, :n])
                nc.gpsimd.tensor_copy(hi0, s[0:64, :n])
                nc.gpsimd.tensor_sub(lo0, s[0:64, :n], hi0)
                nc.vector.tensor_mul(ta[64:128, :n], pb[64:128, :n],
                                     trgc[64:128, cols])
                nc.vector.tensor_mul(tb[64:128, :n], pb[0:64, :n],
                                     trgs[0:64, cols])
                nc.vector.tensor_add(s[64:128, :n], ta[64:128, :n],
                                     tb[64:128, :n])
                nc.gpsimd.tensor_copy(hi1, s[64:128, :n])
                nc.gpsimd.tensor_sub(lo1, s[64:128, :n], hi1)

            def split_evict(ps, hi, lo):
                """hi+lo (fp8) = ps * SCL_EVICT, split across ACT and DVE."""
                nc.scalar.activation(hi, ps, Ident, scale=SCL_EVICT)
                nc.vector.scalar_tensor_tensor(
                    lo, ps, SCL_EVICT, hi, Mult, Sub)

            def proj3(ps, whi, wlo, xhi_ap, xlo_ap, wlo_last=False):
                """ps += 3-term hi/lo fp8 DoubleRow product (contraction WIDTH).

                whi/wlo/xhi_ap/xlo_ap: [128, NW, F] fp8 APs (k-stripe dim 2nd).
                wlo_last orders the lo-weight term last so a still-streaming
                lo-weight load has maximum slack."""
                if wlo_last:
                    terms = ((whi, xhi_ap), (whi, xlo_ap), (wlo, xhi_ap))
                else:
                    terms = ((whi, xhi_ap), (wlo, xhi_ap), (whi, xlo_ap))
                for ti, (wt, xt) in enumerate(terms):
                    for kk in range(NKP):
                        nc.tensor.matmul(
                            ps,
                            lhsT=wt[:, 2 * kk:2 * kk + 2, :],
                            rhs=xt[:, 2 * kk:2 * kk + 2, :],
                            start=(ti == 0 and kk == 0),
                            stop=(ti == 2 and kk == NKP - 1),
                            perf_mode=DR,
                        )

            _ps_pools = [(stp, "st"), (op, "o"), (stp, "st"), (op, "o"),
                         (stp, "st"), (op, "o"), (stp, "st"), (op, "o")]

            def proj_ps(i, cols=QBLK):
                pool, tag = _ps_pools[i % len(_ps_pools)]
                return pool.tile([128, cols], fp32, tag=tag, name=f"ps{i}")

            # ---- Q projection -> rope'd Q^T stripes [qdim, QBLK] ----
            # stripe m: qdim rows [128m, 128m+128) = head m//2, half m%2
            # Only stripes 0..7 (heads 0-3) run up front; the rest stream
            # interleaved into the attention pipeline, where the PE has
            # cover while their wq weights arrive (the projection head of
            # the kernel is DMA-bandwidth-bound, the attention window is
            # DMA-idle).
            def q_evict(m, ps):
                if m % 2 == 0:  # rope half of the head dims
                    rope_evict(ps, slice(0, QBLK),
                               qtr[0:64, 0, m, :], qtr[0:64, 1, m, :],
                               qtr[64:128, 0, m, :], qtr[64:128, 1, m, :])
                else:           # passthrough half
                    split_evict(ps[:], qtr[:, 0, m, :], qtr[:, 1, m, :])

            def q_stripe(m):
                if m + 2 < NW:
                    issue_wq(m + 2)
                wq_m = wq_tiles.pop(m)
                if m in _fences:
                    nc.gpsimd.tensor_copy(dum[:], wq_m[0:1, 0, 0, 0:4])
                    bulk_group(_fences[m])
                ps = proj_ps(m)
                proj3(ps[:], wq_m[:, 0], wq_m[:, 1], xq8[:], xqlo[:])
                q_evict(m, ps)

            # Stripes 0/1: emit the two hi-x terms of both stripes before
            # either stripe's lo-x term, so stripe 1's weight-only work
            # runs while both wait on the (late) xql DMA.
            _pre = []
            for m in range(2):
                issue_wq(m + 2)
                wq_m = wq_tiles.pop(m)
                nc.gpsimd.tensor_copy(dum[:], wq_m[0:1, 0, 0, 0:4])
                bulk_group(_fences[m])
                ps = proj_ps(m)
                for ti, wt in enumerate((wq_m[:, 0], wq_m[:, 1])):
                    for kk in range(NKP):
                        nc.tensor.matmul(
                            ps[:], lhsT=wt[:, 2 * kk:2 * kk + 2, :],
                            rhs=xq8[:, 2 * kk:2 * kk + 2, :],
                            start=(ti == 0 and kk == 0), stop=False,
                            perf_mode=DR)
                _pre.append((wq_m, ps))
            for m in range(2):
                wq_m, ps = _pre[m]
                for kk in range(NKP):
                    nc.tensor.matmul(
                        ps[:], lhsT=wq_m[:, 0, 2 * kk:2 * kk + 2, :],
                        rhs=xqlo[:, 2 * kk:2 * kk + 2, :],
                        start=False, stop=(kk == NKP - 1),
                        perf_mode=DR)
                q_evict(m, ps)

            for m in range(2, NW):
                q_stripe(m)

            # ---- K projection -> rope'd K^T [2, 128, T] fp16 ----
            # Group order consumes col-chunks 0,1 first so the c2/c3
            # loads issued here have time to land.
            _k_iters = [(0, 0), (1, 0), (0, 1), (1, 1),
                        (0, 2), (1, 2), (0, 3), (1, 3)]
            _k_dma = {1: [(1, 1, 0), (1, 1, 1)],
                      2: [(0, 2, 0), (0, 2, 1)], 3: [(1, 2, 0), (1, 2, 1)],
                      4: [(0, 3, 0), (0, 3, 1)], 5: [(1, 3, 0), (1, 3, 1)]}
            for ki, (hh, g) in enumerate(_k_iters):
                    if ki == 0:
                        xf_chunk(0, 1, 1)
                    for ch in _k_dma.get(ki, ()):
                        xf_chunk(*ch)
                    if ki == 6:
                        nc.gpsimd.dma_start(
                            out=wvr[:].rearrange("p s n m -> p s (n m)"),
                            in_=wv[:].rearrange("s p m -> p s m"))
                    elif ki == 7:
                        nc.gpsimd.dma_start(out=masks[:], in_=msk[:])
                        nc.gpsimd.dma_start(out=bia_s[:], in_=bia[:])
                    ps = proj_ps(NW + 4 * hh + g)
                    cols = slice(g * QBLK, (g + 1) * QBLK)
                    kcols = slice(QBLK + g * QBLK, QBLK + (g + 1) * QBLK)
                    proj3(ps[:],
                          wkr[:, 0, :, hh * 128:hh * 128 + 128],
                          wkr[:, 1, :, hh * 128:hh * 128 + 128],
                          x8[:, :, cols], xl[:, :, cols], wlo_last=True)
                    if hh == 0:
                        rope_evict(ps, kcols,
                                   ktr[0:64, 0, 0, cols], ktr[0:64, 1, 0, cols],
                                   ktr[64:128, 0, 0, cols],
                                   ktr[64:128, 1, 0, cols])
                    else:
                        split_evict(ps[:], ktr[:, 0, 1, cols],
                                    ktr[:, 1, 1, cols])

            # ---- V projection: x_hi*wv_hi + x_hi*wv_lo + x_lo*wv_hi ----
            for mt in range(NTT):
                ps = proj_ps(NW + 8 + mt, cols=HEAD_DIM)
                toks = slice(mt * 128, (mt + 1) * 128)
                terms = ((x8, 0), (x8, 1), (xl, 0))
                for ti, (xt, s) in enumerate(terms):
                    for kk in range(NKP):
                        nc.tensor.matmul(
                            ps[:],
                            lhsT=xt[:, 2 * kk:2 * kk + 2, toks],
                            rhs=wvr[:, s, 2 * kk:2 * kk + 2, :],
                            start=(ti == 0 and kk == 0),
                            stop=(ti == 2 and kk == NKP - 1),
                            perf_mode=DR,
                        )
                nc.scalar.activation(
                    vsb[:, mt * VROW: mt * VROW + HEAD_DIM], ps[:], Ident,
                    scale=DESCALE)

            # enc^T (scaled x8, hi/lo fp8) reuses xl's slot
            enct = bigB.tile([128, 2, NW, QBLK], fp8, tag="bigB")

            # ---- attention ----
            # Slot j = q-subtile j (token subtile s+4j): k-tile prefix
            # 0..4j+3 in (j+1) groups of 4 tiles. S^T layout per group:
            # st[:, tt*128+c] = score(k=(4g+tt)*128+p, q=subtile_j col c).
            # Depth-1 software pipeline: QK+exp of slot n is emitted
            # before P@V of slot n-1 so the in-order PE queue always has
            # ready matmuls while exp/mask of the newest slot are in
            # flight on Act/DVE.
            # Head-paired slot order: each j=3 slot sits between j=2/j=3
            # neighbours so its exp chain drains under a long QK+PV cover
            # (a j=0 neighbour would leave the PE waiting ~1us on Act).
            slots = []
            for a in range(NUM_HEADS // 2):
                hA, hB = 2 * a, 2 * a + 1
                slots += [(hA, 2), (hB, 3), (hA, 3), (hB, 2),
                          (hA, 0), (hB, 1), (hA, 1), (hB, 0)]
            pts = {}

            def emit_qk(h, j):
                pt_l = []
                for g in range(j + 1):
                    st = stp.tile([128, QBLK], fp32, tag="st")
                    for tt in range(4):
                        for ti, (ql, kl) in enumerate(
                                ((0, 0), (0, 1), (1, 0))):
                            nc.tensor.matmul(
                                st[:, tt * 128:(tt + 1) * 128],
                                lhsT=ktr[:, kl, 0:2,
                                         (4 * g + tt) * 128:
                                         (4 * g + tt + 1) * 128],
                                rhs=qtr[:, ql, 2 * h:2 * h + 2,
                                        j * 128:(j + 1) * 128],
                                start=(ti == 0), stop=(ti == 2),
                                perf_mode=DR)
                    pt = ptp.tile([128, QBLK], fp16, tag="pt")
                    # p = exp(s / sqrt(head_dim)), masked entries -> 0
                    nc.scalar.activation(pt[:], st[:], Exp,
                                         scale=0.0625 / (S_QK * S_QK))
                    if g == j:  # only the diagonal group needs masking
                        nc.vector.tensor_mul(
                            pt[:], pt[:], masks[:, j * QBLK:(j + 1) * QBLK])
                    pt_l.append(pt)
                pts[(h, j)] = pt_l

            def emit_pv(h, j):
                # Transposed P@V: out^T[hd, q] accumulates directly in the
                # enc^T orientation (lhsT = V tile, rhs = P^T tile), so no
                # PE transposes or PSUM-eviction copies are needed later.
                # One 2KB PSUM bank per slot: enc^T halves [0:256], the
                # denominator column [256] (near-free 1-col matmuls), the
                # transposed 1/denom row [257:385] and its broadcast
                # [384:512]. NOTE: the three accumulation chains must each
                # run their matmuls consecutively — interleaving open
                # accumulation groups within one PSUM bank corrupts them.
                o = op.tile([128, 4 * 128], fp32, tag="o", name=f"o{h}_{j}")
                pt_l = pts.pop((h, j))

                def pt_slice(g, tt):
                    return pt_l[g][:, tt * 128:(tt + 1) * 128]

                for s2 in range(2):
                    for g in range(j + 1):
                        for tt in range(4):
                            t = 4 * g + tt
                            nc.tensor.matmul(
                                o[:, s2 * 128:(s2 + 1) * 128],
                                lhsT=vsb[:, t * VROW + s2 * 128:
                                         t * VROW + (s2 + 1) * 128],
                                rhs=pt_slice(g, tt),
                                start=(g == 0 and tt == 0),
                                stop=(g == j and tt == 3))
                for g in range(j + 1):
                    for tt in range(4):
                        nc.tensor.matmul(
                            o[:, 256:257],
                            lhsT=pt_slice(g, tt),
                            rhs=ones8[:],
                            start=(g == 0 and tt == 0),
                            stop=(g == j and tt == 3))
                r = rcpp.tile([128, 1], fp32, tag="r")
                nc.vector.reciprocal(r[:], o[:, 256:257])
                ors[(h, j)] = (o, r)

            def emit_rt(h, j):
                # stage n-2: transpose 1/denom to a row + stage it in SBUF
                o, r = ors[(h, j)]
                nc.tensor.matmul(o[0:1, 257:385], lhsT=r[:],
                                 rhs=ident32[:], is_transpose=True)
                rts = rcpp.tile([1, 128], fp16, tag="rts", name=f"rs{h}{j}")
                nc.vector.tensor_copy(rts[:], o[0:1, 257:385])
                ors[(h, j)] = (o, rts)

            def emit_fin(h, j):
                # stage n-3: broadcast 1/denom across partitions, scale,
                # and hi/lo fp8 split straight into enc^T.
                o, rts = ors.pop((h, j))
                nc.tensor.matmul(o[:, 384:512], lhsT=ones1[:], rhs=rts[:])
                # TensorTensor may read only one PSUM input (hw verifier):
                # stage the broadcast through SBUF before the scale.
                rbs = enp.tile([128, 128], fp16, tag="rbs", name=f"rb{h}{j}")
                nc.vector.tensor_copy(rbs[:], o[:, 384:512])
                en = enp.tile([128, 2, 128], fp16, tag="en")
                for s2 in range(2):
                    nc.vector.tensor_mul(
                        en[:, s2, :], o[:, s2 * 128:(s2 + 1) * 128],
                        rbs[:])
                # hi/lo fp8 split on Pool, written directly into enc^T
                eh = enct[:, 0, 2 * h:2 * h + 2, j * 128:(j + 1) * 128]
                nc.gpsimd.tensor_copy(eh, en[:])
                nc.gpsimd.tensor_sub(
                    enct[:, 1, 2 * h:2 * h + 2, j * 128:(j + 1) * 128],
                    en[:], eh)

            ors = {}
            wf_tiles = []

            def issue_wf(m):
                wf_m = wqs.tile([128, 2, NW, 128], fp8, tag="wq",
                                name=f"wf{m}")
                nc.sync.dma_start(
                    out=wf_m[:].rearrange("p s n m -> p s (n m)"),
                    in_=wf[m].rearrange("s p m -> p s m"))
                wf_tiles.append(wf_m)

            for n in range(len(slots) + 4):
                if n == len(slots) - 4:
                    # head start on the wf stream while the attention
                    # tail drains (the wqs pool slots are free by now)
                    for m in range(2):
                        issue_wf(m)
                if n < len(slots):
                    emit_qk(*slots[n])
                if 1 <= n <= len(slots):
                    emit_pv(*slots[n - 1])
                if 2 <= n <= len(slots) + 1:
                    emit_rt(*slots[n - 2])
                if 3 <= n <= len(slots) + 2:
                    emit_fin(*slots[n - 3])

            # ---- final projection: out^T = wf @ enc^T + bias ----
            # Pre-issue the whole wf stream on the Pool SWDGE queue; the
            # 4-deep tile pool lets DMAs run ahead of consumption.
            for m in range(2, NW):
                issue_wf(m)
            for m in range(NW):
                wf_m = wf_tiles[m]
                ps = proj_ps(m + 1)
                for ti, (whl, ehl) in enumerate(((0, 0), (1, 0), (0, 1))):
                    for kk in range(NKP):
                        nc.tensor.matmul(
                            ps[:],
                            lhsT=wf_m[:, whl, 2 * kk:2 * kk + 2, :],
                            rhs=enct[:, ehl, 2 * kk:2 * kk + 2, :],
                            start=(ti == 0 and kk == 0),
                            stop=(ti == 2 and kk == NKP - 1),
                            perf_mode=DR,
                        )
                osb = outp.tile([128, QBLK], fp16, tag="osb")
                nc.scalar.activation(osb[:], ps[:], Ident, scale=DESCALE,
                                     bias=bia_s[:, m:m + 1])
                nc.sync.dma_start(out=out[m], in_=osb[:])

    if not nc.is_finalized():
        nc.finalize()  # bacc register allocation — required before walrus compile
    return nc


def get_nc():
    if "nc" not in _NC_CACHE:
        _NC_CACHE["nc"] = _build_nc()
    return _NC_CACHE["nc"]


def _host_prepare(x, segment_pos, wq, wk, wv, w_final, b_final):
    """Build shared + per-core device input arrays."""
    x = np.asarray(x, dtype=np.float32)
    segment_pos = np.asarray(segment_pos)
    wq = np.asarray(wq, dtype=np.float32)
    wk = np.asarray(wk, dtype=np.float32)
    wv = np.asarray(wv, dtype=np.float32)
    w_final = np.asarray(w_final, dtype=np.float32)
    b_final = np.asarray(b_final, dtype=np.float32)

    def hilo(a, s):
        a = a * s
        hi = a.astype(FP8)
        lo = (a - hi.astype(np.float32)).astype(FP8)
        return hi, lo

    def stripes_sq(w):  # [WIDTH, WIDTH] -> [NW,128,WIDTH] w^T stripes (fp32)
        wt = np.ascontiguousarray(w.T)
        return np.ascontiguousarray(
            wt.reshape(NW, 128, NW, 128).transpose(2, 1, 0, 3).reshape(
                NW, 128, WIDTH))

    def skinny(wt):  # [WIDTH, HEAD_DIM] w^T -> [128, NW*HEAD_DIM] (fp32)
        return np.ascontiguousarray(
            wt.reshape(NW, 128, HEAD_DIM).transpose(1, 0, 2).reshape(
                128, NW * HEAD_DIM))

    wq_hi, wq_lo = hilo(stripes_sq(wq), S_W)
    wk_hi, wk_lo = hilo(skinny(np.ascontiguousarray(wk.T)), S_W)
    wv_hi, wv_lo = hilo(skinny(np.ascontiguousarray(wv.T)), S_W)

    shared = {
        "wq": np.ascontiguousarray(np.stack([wq_hi, wq_lo], axis=1)),
        "wk": np.ascontiguousarray(np.stack([wk_hi, wk_lo], axis=0)),
        "wv": np.ascontiguousarray(np.stack([wv_hi, wv_lo], axis=0)),
        "wf": np.ascontiguousarray(np.stack(
            hilo(stripes_sq(w_final), S_W), axis=1)),
        "bia": np.ascontiguousarray(b_final.reshape(NW, 128).T).astype(np.float32),
    }

    inv_freq = (
        1.0 / MAX_WAVELENGTH ** (2.0 * np.arange(HEAD_DIM // 4, dtype=np.float32)
                                 / (HEAD_DIM // 2))
    ).astype(np.float32)

    in_maps = []
    for c in range(8):
        b = c // 4
        s = c % 4
        # core's q tokens: subtiles {s, s+4, s+8, s+12}, 128 each
        qtok = (np.arange(NQS)[:, None] * 4 + s) * 128 + np.arange(128)[None, :]
        qtok = qtok.reshape(-1)  # [QBLK]

        xT = np.ascontiguousarray(x[b].T)  # [WIDTH, T] fp32
        xT_hi, xT_lo = hilo(xT, S_X)
        xfh_ = np.ascontiguousarray(xT_hi.reshape(NW, 128, T))
        xfl_ = np.ascontiguousarray(xT_lo.reshape(NW, 128, T))

        pos = segment_pos[b].astype(np.float32)
        ang_k = inv_freq[:, None] * pos[None, :]          # [64, T]
        ang_q = ang_k[:, qtok]                            # [64, QBLK]
        ang = np.concatenate([ang_q, ang_k], axis=1)      # [64, QBLK+T]
        trc_ = np.concatenate([np.cos(ang)] * 2, axis=0) * SCL_EVICT
        trs_ = np.concatenate([np.sin(ang)] * 2, axis=0) * SCL_EVICT

        # masks for the diagonal 4-tile group of each slot j: k-tiles
        # 4j..4j+3 vs q-subtile s+4j. allow = causal & window & same-seg.
        seg = np.cumsum((segment_pos[b] == 0).astype(np.int64))
        mask_ = np.zeros((128, NQS * QBLK), dtype=np.float32)
        for j in range(NQS):
            tq = (s + 4 * j) * 128 + np.arange(128)       # [128] q tokens
            for tt in range(4):
                tk = (4 * j + tt) * 128 + np.arange(128)  # [128] k tokens
                allow = (
                    (tk[:, None] <= tq[None, :])
                    & (tq[None, :] <= tk[:, None] + WINDOW)
                    & (seg[tk][:, None] == seg[tq][None, :])
                )
                mask_[:, (j * 4 + tt) * 128:(j * 4 + tt + 1) * 128] = allow

        in_maps.append(dict(
            shared,
            xqh=np.ascontiguousarray(xfh_[:, :, qtok]),
            xql=np.ascontiguousarray(xfl_[:, :, qtok]),
            xfh=xfh_,
            xfl=xfl_,
            trc=trc_.astype(FP16),
            trs=trs_.astype(FP16),
            msk=mask_.astype(FP16),
        ))
    return in_maps


def _assemble(results):
    out = np.empty((B, T, WIDTH), dtype=np.float32)
    for c, res in enumerate(results):
        b, s = c // 4, c % 4
        o = np.asarray(res["out"], dtype=np.float32)  # [NW, 128, QBLK]
        o = o.transpose(2, 0, 1).reshape(QBLK, WIDTH)  # [QBLK, WIDTH]
        for j in range(NQS):
            t0 = (s + 4 * j) * 128
            out[b, t0:t0 + 128, :] = o[j * 128:(j + 1) * 128]
    return out


def kernel(x, segment_pos, wq, wk, wv, w_final, b_final):
    from concourse.bass_utils import run_bass_kernel_spmd

    nc = get_nc()
    in_maps = _host_prepare(x, segment_pos, wq, wk, wv, w_final, b_final)
    res = run_bass_kernel_spmd(nc, in_maps, list(range(8)))
    return _assemble(res.results)


# revision 76
# speedup vs baseline: 1.0076x; 1.0018x over previous
"""Trainium2 Bass kernel for a local-attention block (MQA, RoPE, causal mask).

Reference computation (B=2, T=2048, WIDTH=2560, 10 q-heads, 1 kv-head,
head_dim=256, window=2048 => mask reduces to causal & same-segment):

    q = x @ wq.T ; k = x @ wk.T ; v = x @ wv.T
    q, k = rope(q), rope(k)
    probs = softmax(q k^T / 16 + mask)
    out = (probs @ v) @ w_final.T + b_final

Sharding: 8 cores = 2 batches x 4 interleaved query-subtile sets. Core
(b, s) owns q-subtiles {s, s+4, s+8, s+12} (128 tokens each) of batch b.
Slot j (subtile s+4j) needs only the causal k-tile prefix 0..4j+3 — a
FIXED tile count 4*(j+1) on every core, so the device program is
SPMD-uniform while each core skips the k-tiles causality forbids
(400 vs 640 [128x128] attention tile-units per core; ideal is 340).
The <=3-tile overshoot past the causal diagonal is zeroed by a
data-driven 0/1 mask applied only to the last 4-tile group of each
slot; earlier groups are entirely inside the causal region (single
segment; segment_pos is arange per the problem spec).

Precision strategy: projections run as fp8e4m3 DoubleRow matmuls
(0.5 cycles/row) with a host-side hi/lo split of both operands and
three bilinear terms (x_hi*w_hi + x_hi*w_lo + x_lo*w_hi); the dropped
x_lo*w_lo term is ~0.1%. QK^T runs the same 3-term fp8 scheme on
rope'd q/k; P@V runs fp16.
"""

import sys

import numpy as np

for _p in ("/opt/trn_rl_repo", "/root/.axon_site/_ro/trn_rl_repo"):
    if _p not in sys.path:
        sys.path.insert(0, _p)

import ml_dtypes

FP8 = ml_dtypes.float8_e4m3
FP16 = np.float16

B, T, WIDTH = 2, 2048, 2560
NUM_HEADS, HEAD_DIM = 10, 256
WINDOW = 2048
MAX_WAVELENGTH = 10000.0
QBLK = 512              # query tokens per core (4 subtiles of 128)
NW = WIDTH // 128       # 20 width stripes
NKP = NW // 2           # 10 DoubleRow contraction pairs
NTT = T // 128          # 16 token tiles
NQS = QBLK // 128       # 4 query subtiles (slots)
VROW = HEAD_DIM + 1     # v columns + ones column (denominator trick)
# fp8 pre-scales: lift x/w out of the fp8 subnormal floor before hi/lo
# quantization; the product is descaled by DESCALE on eviction.
S_X, S_W = 8.0, 128.0
DESCALE = 1.0 / (S_X * S_W)
S_QK = 8.0              # extra q/k scale lifting their fp8 lo parts
SCL_EVICT = S_QK * DESCALE

_NC_CACHE = {}


def _build_nc():
    """Build the (single, SPMD-uniform) Bass/Tile program."""
    import concourse.bass as bass  # noqa: F401
    import concourse.mybir as mybir
    import concourse.tile as tile
    from concourse import bacc
    from concourse.masks import make_identity

    fp32 = mybir.dt.float32
    fp16 = mybir.dt.float16
    fp8 = mybir.dt.float8e4
    Exp = mybir.ActivationFunctionType.Exp
    Ident = mybir.ActivationFunctionType.Identity
    Mult = mybir.AluOpType.mult
    Sub = mybir.AluOpType.subtract
    DR = mybir.MatmulPerfMode.DoubleRow

    nc = bacc.Bacc("TRN2", target_bir_lowering=False, debug=False)

    # ---- DRAM I/O ----
    # xq*: x^T columns gathered at the core's 4 q-subtiles (hi/lo fp8).
    # xf*: full x^T in natural token order (hi/lo fp8).
    xqh = nc.dram_tensor("xqh", [NW, 128, QBLK], fp8, kind="ExternalInput")
    xql = nc.dram_tensor("xql", [NW, 128, QBLK], fp8, kind="ExternalInput")
    xfh = nc.dram_tensor("xfh", [NW, 128, T], fp8, kind="ExternalInput")
    xfl = nc.dram_tensor("xfl", [NW, 128, T], fp8, kind="ExternalInput")
    wq = nc.dram_tensor("wq", [NW, 2, 128, WIDTH], fp8, kind="ExternalInput")
    wk = nc.dram_tensor("wk", [2, 128, NW * HEAD_DIM], fp8, kind="ExternalInput")
    wv = nc.dram_tensor("wv", [2, 128, NW * HEAD_DIM], fp8, kind="ExternalInput")
    wf = nc.dram_tensor("wf", [NW, 2, 128, WIDTH], fp8, kind="ExternalInput")
    # Cos/sin rope tables, each duplicated across both partition halves
    # (SBUF TensorTensor requires equal input base partitions); cols
    # 0:QBLK q positions (gathered), QBLK: natural k positions.
    trc = nc.dram_tensor("trc", [128, QBLK + T], fp16, kind="ExternalInput")
    trs = nc.dram_tensor("trs", [128, QBLK + T], fp16, kind="ExternalInput")
    # msk: [128 k, (j*4+tt)*128 + c] = 0/1 mask for slot j, group tile
    # 4j+tt, q col c of subtile j.
    msk = nc.dram_tensor("msk", [128, NQS * QBLK], fp16, kind="ExternalInput")
    bia = nc.dram_tensor("bia", [128, NW], fp32, kind="ExternalInput")
    out = nc.dram_tensor("out", [NW, 128, QBLK], fp16, kind="ExternalOutput")

    with tile.TileContext(nc) as tc:
        with (
            tc.tile_pool(name="res", bufs=1) as res,
            tc.tile_pool(name="bigA", bufs=1) as bigA,
            tc.tile_pool(name="bigB", bufs=1) as bigB,
            tc.tile_pool(name="wqs", bufs=4) as wqs,
            tc.tile_pool(name="ptp", bufs=8) as ptp,
            tc.tile_pool(name="enp", bufs=2) as enp,
            tc.tile_pool(name="tmp", bufs=1) as tmpp,
            tc.tile_pool(name="rcp", bufs=2) as rcpp,
            tc.tile_pool(name="outp", bufs=2) as outp,
            tc.tile_pool(name="stp", bufs=4, space="PSUM") as stp,
            tc.tile_pool(name="op", bufs=4, space="PSUM") as op,
        ):
            # ---- resident SBUF tiles ----
            xq8 = res.tile([128, NW, QBLK], fp8, tag="xq8")      # q-col x^T hi
            xqlo = res.tile([128, NW, QBLK], fp8, tag="xqlo")    # q-col x^T lo
            qtr = res.tile([128, 2, NW, QBLK], fp8, tag="qtr")   # rope'd Q^T hi/lo
            ktr = res.tile([128, 2, 2, T], fp8, tag="ktr")       # rope'd K^T hi/lo
            vsb = res.tile([128, NTT * VROW], fp16, tag="vsb")   # V + ones col
            wkr = res.tile([128, 2, NW, HEAD_DIM], fp8, tag="wkr")
            wvr = res.tile([128, 2, NW, HEAD_DIM], fp8, tag="wvr")
            trgc = res.tile([128, QBLK + T], fp16, tag="trgc")
            trgs = res.tile([128, QBLK + T], fp16, tag="trgs")
            masks = res.tile([128, NQS * QBLK], fp16, tag="msk")
            bia_s = res.tile([128, NW], fp32, tag="bia")
            ident = res.tile([128, 128], fp8, tag="ident")
            ones8 = res.tile([128, 1], fp16, tag="ones8")
            ones1 = res.tile([1, 128], fp16, tag="ones1")
            ident32 = res.tile([128, 128], fp32, tag="ident32")

            make_identity(nc, ident[:])
            make_identity(nc, ident32[:])
            nc.gpsimd.memset(ones8[:], 0.125)  # 1/8: bakes enc pre-scale
            nc.gpsimd.memset(ones1[:], 1.0)

            # x^T hi stripes (natural order, full T)
            x8 = bigA.tile([128, NW, T], fp8, tag="bigA")
            # x^T lo stripes; slot later reused for enc^T
            xl = bigB.tile([128, NW, T], fp8, tag="bigB")

            # Fence helper target: a dummy Pool copy keyed on a wq tile
            # paces the bulk Pool-queue loads behind the wq stream.
            dum = res.tile([1, 4], fp8, tag="dum")

            # xq streams in 4 batched halves on the SP queue (HWDGE
            # dispatch is 625ns per DMA); the wq weight stream follows on
            # SP / early stripes on Act so neither blocks the other.
            for half in range(2):  # batched: HWDGE dispatch is 625ns/DMA
                nc.sync.dma_start(
                    out=xq8[:, 10 * half:10 * half + 10, :],
                    in_=xqh[10 * half:10 * half + 10].rearrange(
                        "n p m -> p n m"))
            for a, b in ((0, 6), (6, 10), (10, 14), (14, 20)):
                # pair-aligned chunks: the lo-term streams off each chunk
                # as it lands instead of waiting for the full tensor
                nc.sync.dma_start(
                    out=xqlo[:, a:b, :],
                    in_=xql[a:b].rearrange("n p m -> p n m"))

            wq_tiles = {}

            def issue_wq(m):
                t = wqs.tile([128, 2, NW, 128], fp8, tag="wq")
                # The first stripes ride the Act queue (SP is busy with the
                # xq transfers they must not wait behind); later stripes
                # ride SP, which is free after xq and has no compute ops
                # gating its queue head (Act-queue HOL collapses prefetch).
                eng = nc.scalar if m < 4 else nc.sync
                for s in range(2):  # hi first: first matmuls need only hi
                    eng.dma_start(
                        out=t[:, s].rearrange("p n m -> p (n m)"),
                        in_=wq[m, s])
                wq_tiles[m] = t

            issue_wq(0)
            issue_wq(1)
            # Bulk loads ride the Pool SWDGE queue, split into fenced
            # groups keyed on wq-stripe arrival so they never starve the
            # latency-critical Q-phase weight stream on the shared DMA pool.
            def xf_chunk(hilo, c, half):
                """One [10 stripes, 512 cols] chunk of xfh/xfl (0.66 MB)."""
                a, b = QBLK * c, QBLK * (c + 1)
                n0, n1 = (10, NW) if half else (0, 10)
                src, dst = ((xfh, x8), (xfl, xl))[hilo]
                nc.gpsimd.dma_start(
                    out=dst[:, n0:n1, a:b],
                    in_=src[n0:n1, :, a:b].rearrange("n p m -> p n m"))

            def bulk_group(g):
                if g == 0:
                    nc.gpsimd.dma_start(out=trgc[:], in_=trc[:])
                    nc.gpsimd.dma_start(out=trgs[:], in_=trs[:])
                elif g <= 2:   # xfh col-chunk 0
                    xf_chunk(0, 0, g - 1)
                elif g <= 4:   # xfl col-chunk 0
                    xf_chunk(1, 0, g - 3)
                elif g == 5:   # wk hi half
                    nc.gpsimd.dma_start(
                        out=wkr[:, 0].rearrange("p n m -> p (n m)"),
                        in_=wk[0])
                elif g == 6:   # wk lo half (needed by K iter 1's 3rd term)
                    nc.gpsimd.dma_start(
                        out=wkr[:, 1].rearrange("p n m -> p (n m)"),
                        in_=wk[1])
                else:          # xfh col-chunk 1, first half
                    xf_chunk(0, 1, 0)

            # Only chunks needed at K-phase start are fenced through the
            # Q loop (the Q phase is otherwise DMA-oversubscribed); the
            # c2/c3 chunks stream during the K loop, whose group order is
            # arranged to consume c0/c1 first.
            _fences = {m: m for m in range(8)}

            # denominator columns of V (softmax denom via matmul); 1/8 so
            # the reciprocal bakes in the fp8 enc pre-scale of 8
            for t in range(NTT):
                nc.gpsimd.memset(vsb[:, t * VROW + HEAD_DIM: (t + 1) * VROW], 0.125)

            def rope_evict(ps, cols, hi0, lo0, hi1, lo1):
                """[hi+lo](0) = ps0*cos - ps1*sin ; (1) = ps1*cos + ps0*sin.

                ps: [128, n] PSUM fp32; cols: slice into the trgc/trgs
                tables (carrying the fp8 descale); hi*/lo*: fp8 SBUF APs.
                A single Act cast to fp16 SBUF frees the PSUM slot fast
                (the ps-pool rotation otherwise stalls the PE) and gives
                every DVE mul 2x 16-bit throughput + the cheaper SBUF
                access latency. Each mul pairs inputs from the SAME base
                partition (hw TensorTensor constraint). The hi/lo split
                runs on the otherwise-idle Pool engine."""
                n = cols.stop - cols.start
                pb = tmpp.tile([128, QBLK], fp16, tag="pb", name="pb")
                ta = tmpp.tile([128, QBLK], fp16, tag="ta", name="ta")
                tb = tmpp.tile([128, QBLK], fp16, tag="tb", name="tb")
                s = tmpp.tile([128, QBLK], fp16, tag="s", name="s")
                nc.scalar.activation(pb[:, :n], ps[:], Ident)
                nc.vector.tensor_mul(ta[0:64, :n], pb[0:64, :n],
                                     trgc[0:64, cols])
                nc.vector.tensor_mul(tb[0:64, :n], pb[64:128, :n],
                                     trgs[64:128, cols])
                nc.vector.tensor_sub(s[0:64, :n], ta[0:64, :n], tb[0:64, :n])
                nc.gpsimd.tensor_copy(hi0, s[0:64, :n])
                nc.gpsimd.tensor_sub(lo0, s[0:64, :n], hi0)
                nc.vector.tensor_mul(ta[64:128, :n], pb[64:128, :n],
                                     trgc[64:128, cols])
                nc.vector.tensor_mul(tb[64:128, :n], pb[0:64, :n],
                                     trgs[0:64, cols])
                nc.vector.tensor_add(s[64:128, :n], ta[64:128, :n],
                                     tb[64:128, :n])
                nc.gpsimd.tensor_copy(hi1, s[64:128, :n])
                nc.gpsimd.tensor_sub(lo1, s[64:128, :n], hi1)

            def split_evict(ps, hi, lo):
                """hi+lo (fp8) = ps * SCL_EVICT, split across ACT and DVE."""
                nc.scalar.activation(hi, ps, Ident, scale=SCL_EVICT)
                nc.vector.scalar_tensor_tensor(
                    lo, ps, SCL_EVICT, hi, Mult, Sub)

            def proj3(ps, whi, wlo, xhi_ap, xlo_ap, wlo_last=False):
                """ps += 3-term hi/lo fp8 DoubleRow product (contraction WIDTH).

                whi/wlo/xhi_ap/xlo_ap: [128, NW, F] fp8 APs (k-stripe dim 2nd).
                wlo_last orders the lo-weight term last so a still-streaming
                lo-weight load has maximum slack."""
                if wlo_last:
                    terms = ((whi, xhi_ap), (whi, xlo_ap), (wlo, xhi_ap))
                else:
                    terms = ((whi, xhi_ap), (wlo, xhi_ap), (whi, xlo_ap))
                for ti, (wt, xt) in enumerate(terms):
                    for kk in range(NKP):
                        nc.tensor.matmul(
                            ps,
                            lhsT=wt[:, 2 * kk:2 * kk + 2, :],
                            rhs=xt[:, 2 * kk:2 * kk + 2, :],
                            start=(ti == 0 and kk == 0),
                            stop=(ti == 2 and kk == NKP - 1),
                            perf_mode=DR,
                        )

            _ps_pools = [(stp, "st"), (op, "o"), (stp, "st"), (op, "o"),
                         (stp, "st"), (op, "o"), (stp, "st"), (op, "o")]

            def proj_ps(i, cols=QBLK):
                pool, tag = _ps_pools[i % len(_ps_pools)]
                return pool.tile([128, cols], fp32, tag=tag, name=f"ps{i}")

            # ---- Q projection -> rope'd Q^T stripes [qdim, QBLK] ----
            # stripe m: qdim rows [128m, 128m+128) = head m//2, half m%2
            # Only stripes 0..7 (heads 0-3) run up front; the rest stream
            # interleaved into the attention pipeline, where the PE has
            # cover while their wq weights arrive (the projection head of
            # the kernel is DMA-bandwidth-bound, the attention window is
            # DMA-idle).
            def q_evict(m, ps):
                if m % 2 == 0:  # rope half of the head dims
                    rope_evict(ps, slice(0, QBLK),
                               qtr[0:64, 0, m, :], qtr[0:64, 1, m, :],
                               qtr[64:128, 0, m, :], qtr[64:128, 1, m, :])
                else:           # passthrough half
                    split_evict(ps[:], qtr[:, 0, m, :], qtr[:, 1, m, :])

            def q_stripe(m):
                if m + 2 < NW:
                    issue_wq(m + 2)
                wq_m = wq_tiles.pop(m)
                if m in _fences:
                    nc.gpsimd.tensor_copy(dum[:], wq_m[0:1, 0, 0, 0:4])
                    bulk_group(_fences[m])
                ps = proj_ps(m)
                proj3(ps[:], wq_m[:, 0], wq_m[:, 1], xq8[:], xqlo[:])
                q_evict(m, ps)

            # Stripes 0/1: emit the two hi-x terms of both stripes before
            # either stripe's lo-x term, so stripe 1's weight-only work
            # runs while both wait on the (late) xql DMA.
            _pre = []
            for m in range(2):
                issue_wq(m + 2)
                wq_m = wq_tiles.pop(m)
                nc.gpsimd.tensor_copy(dum[:], wq_m[0:1, 0, 0, 0:4])
                bulk_group(_fences[m])
                ps = proj_ps(m)
                for ti, wt in enumerate((wq_m[:, 0], wq_m[:, 1])):
                    for kk in range(NKP):
                        nc.tensor.matmul(
                            ps[:], lhsT=wt[:, 2 * kk:2 * kk + 2, :],
                            rhs=xq8[:, 2 * kk:2 * kk + 2, :],
                            start=(ti == 0 and kk == 0), stop=False,
                            perf_mode=DR)
                _pre.append((wq_m, ps))
            for m in range(2):
                wq_m, ps = _pre[m]
                for kk in range(NKP):
                    nc.tensor.matmul(
                        ps[:], lhsT=wq_m[:, 0, 2 * kk:2 * kk + 2, :],
                        rhs=xqlo[:, 2 * kk:2 * kk + 2, :],
                        start=False, stop=(kk == NKP - 1),
                        perf_mode=DR)
                q_evict(m, ps)

            for m in range(2, NW):
                q_stripe(m)

            # ---- K projection -> rope'd K^T [2, 128, T] fp16 ----
            # Group order consumes col-chunks 0,1 first so the c2/c3
            # loads issued here have time to land.
            _k_iters = [(0, 0), (1, 0), (0, 1), (1, 1),
                        (0, 2), (1, 2), (0, 3), (1, 3)]
            _k_dma = {1: [(1, 1, 0), (1, 1, 1)],
                      2: [(0, 2, 0), (0, 2, 1)], 3: [(1, 2, 0), (1, 2, 1)],
                      4: [(0, 3, 0), (0, 3, 1)], 5: [(1, 3, 0), (1, 3, 1)]}
            for ki, (hh, g) in enumerate(_k_iters):
                    if ki == 0:
                        xf_chunk(0, 1, 1)
                    for ch in _k_dma.get(ki, ()):
                        xf_chunk(*ch)
                    if ki == 6:
                        nc.gpsimd.dma_start(
                            out=wvr[:].rearrange("p s n m -> p s (n m)"),
                            in_=wv[:].rearrange("s p m -> p s m"))
                    elif ki == 7:
                        nc.gpsimd.dma_start(out=masks[:], in_=msk[:])
                        nc.gpsimd.dma_start(out=bia_s[:], in_=bia[:])
                    ps = proj_ps(NW + 4 * hh + g)
                    cols = slice(g * QBLK, (g + 1) * QBLK)
                    kcols = slice(QBLK + g * QBLK, QBLK + (g + 1) * QBLK)
                    proj3(ps[:],
                          wkr[:, 0, :, hh * 128:hh * 128 + 128],
                          wkr[:, 1, :, hh * 128:hh * 128 + 128],
                          x8[:, :, cols], xl[:, :, cols], wlo_last=True)
                    if hh == 0:
                        rope_evict(ps, kcols,
                                   ktr[0:64, 0, 0, cols], ktr[0:64, 1, 0, cols],
                                   ktr[64:128, 0, 0, cols],
                                   ktr[64:128, 1, 0, cols])
                    else:
                        split_evict(ps[:], ktr[:, 0, 1, cols],
                                    ktr[:, 1, 1, cols])

            # ---- V projection: x_hi*wv_hi + x_hi*wv_lo + x_lo*wv_hi ----
            for mt in range(NTT):
                ps = proj_ps(NW + 8 + mt, cols=HEAD_DIM)
                toks = slice(mt * 128, (mt + 1) * 128)
                terms = ((x8, 0), (x8, 1), (xl, 0))
                for ti, (xt, s) in enumerate(terms):
                    for kk in range(NKP):
                        nc.tensor.matmul(
                            ps[:],
                            lhsT=xt[:, 2 * kk:2 * kk + 2, toks],
                            rhs=wvr[:, s, 2 * kk:2 * kk + 2, :],
                            start=(ti == 0 and kk == 0),
                            stop=(ti == 2 and kk == NKP - 1),
                            perf_mode=DR,
                        )
                nc.scalar.activation(
                    vsb[:, mt * VROW: mt * VROW + HEAD_DIM], ps[:], Ident,
                    scale=DESCALE)

            # enc^T (scaled x8, hi/lo fp8) reuses xl's slot
            enct = bigB.tile([128, 2, NW, QBLK], fp8, tag="bigB")

            # ---- attention ----
            # Slot j = q-subtile j (token subtile s+4j): k-tile prefix
            # 0..4j+3 in (j+1) groups of 4 tiles. S^T layout per group:
            # st[:, tt*128+c] = score(k=(4g+tt)*128+p, q=subtile_j col c).
            # Depth-1 software pipeline: QK+exp of slot n is emitted
            # before P@V of slot n-1 so the in-order PE queue always has
            # ready matmuls while exp/mask of the newest slot are in
            # flight on Act/DVE.
            # Head-paired slot order: each j=3 slot sits between j=2/j=3
            # neighbours so its exp chain drains under a long QK+PV cover
            # (a j=0 neighbour would leave the PE waiting ~1us on Act).
            slots = []
            for a in range(NUM_HEADS // 2):
                hA, hB = 2 * a, 2 * a + 1
                slots += [(hA, 2), (hB, 3), (hA, 3), (hB, 2),
                          (hA, 0), (hB, 1), (hA, 1), (hB, 0)]
            pts = {}

            def emit_qk(h, j):
                pt_l = []
                for g in range(j + 1):
                    st = stp.tile([128, QBLK], fp32, tag="st")
                    for tt in range(4):
                        for ti, (ql, kl) in enumerate(
                                ((0, 0), (0, 1), (1, 0))):
                            nc.tensor.matmul(
                                st[:, tt * 128:(tt + 1) * 128],
                                lhsT=ktr[:, kl, 0:2,
                                         (4 * g + tt) * 128:
                                         (4 * g + tt + 1) * 128],
                                rhs=qtr[:, ql, 2 * h:2 * h + 2,
                                        j * 128:(j + 1) * 128],
                                start=(ti == 0), stop=(ti == 2),
                                perf_mode=DR)
                    pt = ptp.tile([128, QBLK], fp16, tag="pt")
                    # p = exp(s / sqrt(head_dim)), masked entries -> 0
                    nc.scalar.activation(pt[:], st[:], Exp,
                                         scale=0.0625 / (S_QK * S_QK))
                    if g == j:  # only the diagonal group needs masking
                        nc.vector.tensor_mul(
                            pt[:], pt[:], masks[:, j * QBLK:(j + 1) * QBLK])
                    pt_l.append(pt)
                pts[(h, j)] = pt_l

            def emit_pv(h, j):
                # Transposed P@V: out^T[hd, q] accumulates directly in the
                # enc^T orientation (lhsT = V tile, rhs = P^T tile), so no
                # PE transposes or PSUM-eviction copies are needed later.
                # One 2KB PSUM bank per slot: enc^T halves [0:256], the
                # denominator column [256] (near-free 1-col matmuls), the
                # transposed 1/denom row [257:385] and its broadcast
                # [384:512]. NOTE: the three accumulation chains must each
                # run their matmuls consecutively — interleaving open
                # accumulation groups within one PSUM bank corrupts them.
                o = op.tile([128, 4 * 128], fp32, tag="o", name=f"o{h}_{j}")
                pt_l = pts.pop((h, j))

                def pt_slice(g, tt):
                    return pt_l[g][:, tt * 128:(tt + 1) * 128]

                for s2 in range(2):
                    for g in range(j + 1):
                        for tt in range(4):
                            t = 4 * g + tt
                            nc.tensor.matmul(
                                o[:, s2 * 128:(s2 + 1) * 128],
                                lhsT=vsb[:, t * VROW + s2 * 128:
                                         t * VROW + (s2 + 1) * 128],
                                rhs=pt_slice(g, tt),
                                start=(g == 0 and tt == 0),
                                stop=(g == j and tt == 3))
                for g in range(j + 1):
                    for tt in range(4):
                        nc.tensor.matmul(
                            o[:, 256:257],
                            lhsT=pt_slice(g, tt),
                            rhs=ones8[:],
                            start=(g == 0 and tt == 0),
                            stop=(g == j and tt == 3))
                r = rcpp.tile([128, 1], fp32, tag="r")
                nc.vector.reciprocal(r[:], o[:, 256:257])
                ors[(h, j)] = (o, r)

            def emit_rt(h, j):
                # stage n-2: transpose 1/denom to a row + stage it in SBUF
                o, r = ors[(h, j)]
                nc.tensor.matmul(o[0:1, 257:385], lhsT=r[:],
                                 rhs=ident32[:], is_transpose=True)
                rts = rcpp.tile([1, 128], fp16, tag="rts", name=f"rs{h}{j}")
                nc.vector.tensor_copy(rts[:], o[0:1, 257:385])
                ors[(h, j)] = (o, rts)

            def emit_fin(h, j):
                # stage n-3: broadcast 1/denom across partitions, scale,
                # and hi/lo fp8 split straight into enc^T.
                o, rts = ors.pop((h, j))
                nc.tensor.matmul(o[:, 384:512], lhsT=ones1[:], rhs=rts[:])
                # TensorTensor may read only one PSUM input (hw verifier):
                # stage the broadcast through SBUF before the scale.
                rbs = enp.tile([128, 128], fp16, tag="rbs", name=f"rb{h}{j}")
                nc.vector.tensor_copy(rbs[:], o[:, 384:512])
                en = enp.tile([128, 2, 128], fp16, tag="en")
                for s2 in range(2):
                    nc.vector.tensor_mul(
                        en[:, s2, :], o[:, s2 * 128:(s2 + 1) * 128],
                        rbs[:])
                # hi/lo fp8 split on Pool, written directly into enc^T
                eh = enct[:, 0, 2 * h:2 * h + 2, j * 128:(j + 1) * 128]
                nc.gpsimd.tensor_copy(eh, en[:])
                nc.gpsimd.tensor_sub(
                    enct[:, 1, 2 * h:2 * h + 2, j * 128:(j + 1) * 128],
                    en[:], eh)

            ors = {}
            wf_tiles = []

            def issue_wf(m):
                wf_m = wqs.tile([128, 2, NW, 128], fp8, tag="wq",
                                name=f"wf{m}")
                nc.sync.dma_start(
                    out=wf_m[:].rearrange("p s n m -> p s (n m)"),
                    in_=wf[m].rearrange("s p m -> p s m"))
                wf_tiles.append(wf_m)

            for n in range(len(slots) + 4):
                if n == len(slots) - 4:
                    # head start on the wf stream while the attention
                    # tail drains (the wqs pool slots are free by now)
                    for m in range(2):
                        issue_wf(m)
                if n < len(slots):
                    emit_qk(*slots[n])
                if 1 <= n <= len(slots):
                    emit_pv(*slots[n - 1])
                if 2 <= n <= len(slots) + 1:
                    emit_rt(*slots[n - 2])
                if 3 <= n <= len(slots) + 2:
                    emit_fin(*slots[n - 3])

            # ---- final projection: out^T = wf @ enc^T + bias ----
            # Pre-issue the whole wf stream on the Pool SWDGE queue; the
            # 4-deep tile pool lets DMAs run ahead of consumption.
            for m in range(2, NW):
                issue_wf(m)
            for m in range(NW):
                wf_m = wf_tiles[m]
                ps = proj_ps(m + 1)
                for ti, (whl, ehl) in enumerate(((0, 0), (1, 0), (0, 1))):
                    for kk in range(NKP):
                        nc.tensor.matmul(
                            ps[:],
                            lhsT=wf_m[:, whl, 2 * kk:2 * kk + 2, :],
                            rhs=enct[:, ehl, 2 * kk:2 * kk + 2, :],
                            start=(ti == 0 and kk == 0),
                            stop=(ti == 2 and kk == NKP - 1),
                            perf_mode=DR,
                        )
                osb = outp.tile([128, QBLK], fp16, tag="osb")
                nc.scalar.activation(osb[:], ps[:], Ident, scale=DESCALE,
                                     bias=bia_s[:, m:m + 1])
                nc.sync.dma_start(out=out[m], in_=osb[:])

    if not nc.is_finalized():
        nc.finalize()  # bacc register allocation — required before walrus compile
    return nc


def get_nc():
    if "nc" not in _NC_CACHE:
        _NC_CACHE["nc"] = _build_nc()
    return _NC_CACHE["nc"]


def _host_prepare(x, segment_pos, wq, wk, wv, w_final, b_final):
    """Build shared + per-core device input arrays."""
    x = np.asarray(x, dtype=np.float32)
    segment_pos = np.asarray(segment_pos)
    wq = np.asarray(wq, dtype=np.float32)
    wk = np.asarray(wk, dtype=np.float32)
    wv = np.asarray(wv, dtype=np.float32)
    w_final = np.asarray(w_final, dtype=np.float32)
    b_final = np.asarray(b_final, dtype=np.float32)

    def hilo(a, s):
        a = a * s
        hi = a.astype(FP8)
        lo = (a - hi.astype(np.float32)).astype(FP8)
        return hi, lo

    def stripes_sq(w):  # [WIDTH, WIDTH] -> [NW,128,WIDTH] w^T stripes (fp32)
        wt = np.ascontiguousarray(w.T)
        return np.ascontiguousarray(
            wt.reshape(NW, 128, NW, 128).transpose(2, 1, 0, 3).reshape(
                NW, 128, WIDTH))

    def skinny(wt):  # [WIDTH, HEAD_DIM] w^T -> [128, NW*HEAD_DIM] (fp32)
        return np.ascontiguousarray(
            wt.reshape(NW, 128, HEAD_DIM).transpose(1, 0, 2).reshape(
                128, NW * HEAD_DIM))

    wq_hi, wq_lo = hilo(stripes_sq(wq), S_W)
    wk_hi, wk_lo = hilo(skinny(np.ascontiguousarray(wk.T)), S_W)
    wv_hi, wv_lo = hilo(skinny(np.ascontiguousarray(wv.T)), S_W)

    shared = {
        "wq": np.ascontiguousarray(np.stack([wq_hi, wq_lo], axis=1)),
        "wk": np.ascontiguousarray(np.stack([wk_hi, wk_lo], axis=0)),
        "wv": np.ascontiguousarray(np.stack([wv_hi, wv_lo], axis=0)),
        "wf": np.ascontiguousarray(np.stack(
            hilo(stripes_sq(w_final), S_W), axis=1)),
        "bia": np.ascontiguousarray(b_final.reshape(NW, 128).T).astype(np.float32),
    }

    inv_freq = (
        1.0 / MAX_WAVELENGTH ** (2.0 * np.arange(HEAD_DIM // 4, dtype=np.float32)
                                 / (HEAD_DIM // 2))
    ).astype(np.float32)

    in_maps = []
    for c in range(8):
        b = c // 4
        s = c % 4
        # core's q tokens: subtiles {s, s+4, s+8, s+12}, 128 each
        qtok = (np.arange(NQS)[:, None] * 4 + s) * 128 + np.arange(128)[None, :]
        qtok = qtok.reshape(-1)  # [QBLK]

        xT = np.ascontiguousarray(x[b].T)  # [WIDTH, T] fp32
        xT_hi, xT_lo = hilo(xT, S_X)
        xfh_ = np.ascontiguousarray(xT_hi.reshape(NW, 128, T))
        xfl_ = np.ascontiguousarray(xT_lo.reshape(NW, 128, T))

        pos = segment_pos[b].astype(np.float32)
        ang_k = inv_freq[:, None] * pos[None, :]          # [64, T]
        ang_q = ang_k[:, qtok]                            # [64, QBLK]
        ang = np.concatenate([ang_q, ang_k], axis=1)      # [64, QBLK+T]
        trc_ = np.concatenate([np.cos(ang)] * 2, axis=0) * SCL_EVICT
        trs_ = np.concatenate([np.sin(ang)] * 2, axis=0) * SCL_EVICT

        # masks for the diagonal 4-tile group of each slot j: k-tiles
        # 4j..4j+3 vs q-subtile s+4j. allow = causal & window & same-seg.
        seg = np.cumsum((segment_pos[b] == 0).astype(np.int64))
        mask_ = np.zeros((128, NQS * QBLK), dtype=np.float32)
        for j in range(NQS):
            tq = (s + 4 * j) * 128 + np.arange(128)       # [128] q tokens
            for tt in range(4):
                tk = (4 * j + tt) * 128 + np.arange(128)  # [128] k tokens
                allow = (
                    (tk[:, None] <= tq[None, :])
                    & (tq[None, :] <= tk[:, None] + WINDOW)
                    & (seg[tk][:, None] == seg[tq][None, :])
                )
                mask_[:, (j * 4 + tt) * 128:(j * 4 + tt + 1) * 128] = allow

        in_maps.append(dict(
            shared,
            xqh=np.ascontiguousarray(xfh_[:, :, qtok]),
            xql=np.ascontiguousarray(xfl_[:, :, qtok]),
            xfh=xfh_,
            xfl=xfl_,
            trc=trc_.astype(FP16),
            trs=trs_.astype(FP16),
            msk=mask_.astype(FP16),
        ))
    return in_maps


def _assemble(results):
    out = np.empty((B, T, WIDTH), dtype=np.float32)
    for c, res in enumerate(results):
        b, s = c // 4, c % 4
        o = np.asarray(res["out"], dtype=np.float32)  # [NW, 128, QBLK]
        o = o.transpose(2, 0, 1).reshape(QBLK, WIDTH)  # [QBLK, WIDTH]
        for j in range(NQS):
            t0 = (s + 4 * j) * 128
            out[b, t0:t0 + 128, :] = o[j * 128:(j + 1) * 128]
    return out


def kernel(x, segment_pos, wq, wk, wv, w_final, b_final):
    from concourse.bass_utils import run_bass_kernel_spmd

    nc = get_nc()
    in_maps = _host_prepare(x, segment_pos, wq, wk, wv, w_final, b_final)
    res = run_bass_kernel_spmd(nc, in_maps, list(range(8)))
    return _assemble(res.results)
